# revision 1
# baseline (speedup 1.0000x reference)
"""GCNConv (PyG semantics) on 8 Trainium2 NeuronCores.

out = D^-1/2 (A+I) D^-1/2 (x @ W.T) + b, dst-sharded across 8 cores.

Host: bucket edges (plus self loops) by dst shard; split each node's edge
list by 4 source windows of 32,767 rows (dma_gather indices are int16); pad
each (node, window) run into 4-edge slots; pack slots into 128-edge chunks
(32 slots/chunk).

Device, per core:
  Phase A: h' = (x @ W.T) * dinv[row] over a 131072-row padded table
           (1 zero row per window); written to HBM.
  Phase B: per window, dma_gather 2048 messages at a time from h'; PE matmul
           with a constant slot-prefix matrix SP32 [128,32] -> PSUM [32,512]
           (slot-prefix sums); DVE/ACT copy to SBUF; DMA-stage to a DRAM
           slot-prefix table (row 0 reserved zero).
  Phase C: per window, dma_gather per-node boundary prefix rows (a = last
           slot, b = before first slot), acc += A_w - B_w; then scale by
           dinv[dst], add bias, write out shard.
"""

import numpy as np
from contextlib import ExitStack

import concourse.bacc as bacc
import concourse.bass as bass
import concourse.mybir as mybir
from concourse import bass_utils
from concourse.library_config import mlp

D = 64
L = 8                        # slot length (edges)
SPC = 16                     # slots per 128-edge chunk
GCH = 16                     # chunks per dma_gather
NIDX = GCH * 128             # 2048 idxs per gather


def configure(n=100000, ncores=8, wcap=32767, nw=4, l2g=1792):
    # set problem geometry (module globals); defaults = real problem
    global N, NCORES, SHARD, WCAP, NW, WSTRIDE, TBL, ZROW
    global OUTROWS, NCOLS, L2G, L2K, AIT
    N = n
    NCORES = ncores
    SHARD = N // NCORES
    WCAP = wcap
    NW = nw
    WSTRIDE = WCAP + 1
    assert WSTRIDE % 128 == 0 and NW * WCAP >= N
    TBL = NW * WSTRIDE
    ZROW = WCAP
    OUTROWS = -(-SHARD // 128) * 128
    NCOLS = OUTROWS // 128
    L2G = l2g
    assert OUTROWS % L2G == 0 and L2G % 128 == 0
    L2K = OUTROWS // L2G
    assert TBL % 1024 == 0
    AIT = TBL // 1024
    global AIT_REAL
    max_row = (N - 1) + (N - 1) // WCAP
    AIT_REAL = -(-(max_row + 1) // 1024)


configure()
LAST_NC = None


def _wrap16(idx_flat):
    """Flat idx list -> dma_gather int16 wrap [16, n//16] tiled to 128."""
    n = idx_flat.shape[0]
    out = idx_flat.reshape(n // 16, 16).T.astype(np.int16)
    return np.tile(out, (8, 1))


def _prep_core(src_g, dst_l):
    """Per-core, per-window gather/aggregation structures."""
    win = src_g // WCAP
    loc = src_g - win * WCAP  # 0..32766

    res = {"idx": [], "a": [], "b": [], "C": []}
    for w in range(NW):
        m = win == w
        dw = dst_l[m]
        lw = loc[m]
        order = np.argsort(dw, kind="stable")
        dw = dw[order]
        lw = lw[order]
        counts = np.bincount(dw, minlength=SHARD)
        slots = -(-counts // L)
        slot_start = np.zeros(SHARD, np.int64)
        chunk_of = np.zeros(SHARD, np.int64)
        cur_chunk, cur_slot = 0, 0
        for n_ in range(SHARD):
            s = slots[n_]
            if s == 0:
                continue
            if s > SPC:
                raise ValueError(f"node needs {s} slots > {SPC}")
            if cur_slot + s > SPC:
                cur_chunk += 1
                cur_slot = 0
            chunk_of[n_] = cur_chunk
            slot_start[n_] = cur_slot
            cur_slot += s
        C = cur_chunk + (1 if cur_slot > 0 else 0)
        nz = counts > 0
        starts = np.zeros(SHARD, np.int64)
        starts[1:] = np.cumsum(counts)[:-1]
        pos_base = chunk_of * 128 + slot_start * L
        idx = np.full(max(C, 1) * 128, ZROW, np.int64)
        within = np.arange(dw.shape[0]) - np.repeat(starts[nz], counts[nz])
        pos = np.repeat(pos_base[nz], counts[nz]) + within
        idx[pos] = lw
        a = np.zeros(SHARD, np.int64)
        b = np.zeros(SHARD, np.int64)

        def sprow(ch, sl):
            return (ch // GCH) * (GCH * SPC) + sl * GCH + ch % GCH + 1

        a[nz] = sprow(chunk_of[nz], slot_start[nz] + slots[nz] - 1)
        sb0 = slot_start[nz] > 0
        bnz = np.zeros(int(nz.sum()), np.int64)
        bnz[sb0] = sprow(chunk_of[nz][sb0], slot_start[nz][sb0] - 1)
        b[nz] = bnz
        res["idx"].append(idx)
        res["a"].append(a)
        res["b"].append(b)
        res["C"].append(C)
    return res


def _build_program(C1, SPROWS, G1s):
    sched = [(w, gg) for w in range(NW) for gg in range(G1s[w])]
    NG = len(sched)             # total phase-B groups
    Gsum = [0]
    for w in range(NW):
        Gsum.append(Gsum[-1] + G1s[w])
    dt = mybir.dt
    TCOL = TBL // 128           # 1024
    NAB = OUTROWS // 16         # 784 idx cols per (window, a|b)

    nc = bacc.Bacc("TRN2", target_bir_lowering=False, debug=False,
                   num_devices=NCORES)
    t_xT = nc.dram_tensor("xT", [D, TBL], dt.float32, kind="ExternalInput")
    t_WT = nc.dram_tensor("WT", [D, D], dt.float32, kind="ExternalInput")
    t_SP = nc.dram_tensor("SP", [128, SPC], dt.float32, kind="ExternalInput")
    t_degT = nc.dram_tensor("degT", [128, TCOL], dt.float32,
                            kind="ExternalInput")
    t_degD = nc.dram_tensor("degD", [128, NCOLS], dt.float32,
                            kind="ExternalInput")
    t_bBC = nc.dram_tensor("bBC", [128, D], dt.float32, kind="ExternalInput")
    t_idx1 = nc.dram_tensor("idx1", [NW, 128, C1 * 8], dt.int16,
                            kind="ExternalInput")
    t_idxa = nc.dram_tensor("idxa", [128, NW * NAB], dt.int16,
                            kind="ExternalInput")
    t_idxb = nc.dram_tensor("idxb", [128, NW * NAB], dt.int16,
                            kind="ExternalInput")
    t_hp = nc.dram_tensor("hp", [TBL, D], dt.float32)
    t_sp = nc.dram_tensor("sp", [NW, SPROWS, D], dt.float32)
    t_out = nc.dram_tensor("out_s", [OUTROWS, D], dt.float32,
                           kind="ExternalOutput")

    with ExitStack() as ctx:
        e = ctx.enter_context
        xb = [e(nc.sbuf_tensor(f"xb{i}", [D, 1024], dt.float32))
              for i in range(4)]
        hb = [e(nc.sbuf_tensor(f"hb{i}", [128, 512], dt.float32))
              for i in range(4)]
        WTs = e(nc.sbuf_tensor("WTs", [D, D], dt.float32))
        SPs = e(nc.sbuf_tensor("SPs", [128, SPC], dt.float32))
        bBCs = e(nc.sbuf_tensor("bBCs", [128, D], dt.float32))
        degTs = e(nc.sbuf_tensor("degTs", [128, TCOL], dt.float32))
        dinvTs = e(nc.sbuf_tensor("dinvTs", [128, TCOL], dt.float32))
        degDs = e(nc.sbuf_tensor("degDs", [128, NCOLS], dt.float32))
        dinvDs = e(nc.sbuf_tensor("dinvDs", [128, NCOLS], dt.float32))
        idx1s = e(nc.sbuf_tensor("idx1s", [128, C1 * 8], dt.int16))
        idxas = e(nc.sbuf_tensor("idxas", [128, NW * NAB], dt.int16))
        idxbs = e(nc.sbuf_tensor("idxbs", [128, NW * NAB], dt.int16))
        msg = [e(nc.sbuf_tensor(f"msg{i}", [128, GCH, D], dt.float32))
               for i in range(4)]
        sps = [e(nc.sbuf_tensor(f"sps{i}", [SPC, GCH * D], dt.float32))
               for i in range(4)]
        zrow = e(nc.sbuf_tensor("zrow", [1, D], dt.float32))
        Ab = e(nc.sbuf_tensor("Ab", [128, NCOLS, D], dt.float32))
        Bb = e(nc.sbuf_tensor("Bb", [128, NCOLS, D], dt.float32))
        accs = e(nc.sbuf_tensor("accs", [128, NCOLS, D], dt.float32))
        psum = [e(nc.psum_tensor(f"ps{i}", [128, 512], dt.float32))
                for i in range(8)]

        sLD = e(nc.semaphore("sLD"))
        sAx = [e(nc.semaphore(f"sAx{i}")) for i in range(4)]
        sAmm = e(nc.semaphore("sAmm"))
        sAsc = e(nc.semaphore("sAsc"))
        sAout = [e(nc.semaphore(f"sAout{i}")) for i in range(4)]
        sDin = e(nc.semaphore("sDin"))
        sBidx = e(nc.semaphore("sBidx"))
        sBg = [e(nc.semaphore(f"sBg{i}")) for i in range(4)]
        sBmm = e(nc.semaphore("sBmm"))
        sBcpV = e(nc.semaphore("sBcpV"))
        sBcpS = e(nc.semaphore("sBcpS"))
        sBst = [e(nc.semaphore(f"sBst{i}")) for i in range(4)]
        sCz = e(nc.semaphore("sCz"))
        sCa = e(nc.semaphore("sCa"))
        sCb = e(nc.semaphore("sCb"))
        sCacc = e(nc.semaphore("sCacc"))
        sFin = e(nc.semaphore("sFin"))

        def bcast(ap, reps):
            return bass.AP(ap.tensor, ap.offset, list(ap.ap) + [[0, reps]])

        with nc.Block() as block:

            @block.sync
            def _(sync: bass.BassEngine):
                sync.dma_start(WTs[:], t_WT[:]).then_inc(sLD, 16)
                sync.dma_start(SPs[:], t_SP[:]).then_inc(sLD, 16)
                sync.dma_start(bBCs[:], t_bBC[:]).then_inc(sLD, 16)
                sync.dma_start(degTs[:], t_degT[:]).then_inc(sLD, 16)
                sync.dma_start(degDs[:], t_degD[:]).then_inc(sLD, 16)
                sync.dma_start(idxas[:], t_idxa[:]).then_inc(sLD, 16)
                sync.dma_start(idxbs[:], t_idxb[:]).then_inc(sLD, 16)
                # phase A, interleaved x-in / h'-out
                for it in range(AIT_REAL + 2):
                    if it < AIT_REAL:
                        if it >= 4:
                            sync.wait_ge(sAmm, it - 3)
                        sync.dma_start(
                            xb[it % 4][:], t_xT[:, it * 1024:(it + 1) * 1024]
                        ).then_inc(sAx[it % 4], 16)
                    if it >= 2:
                        jo = it - 2
                        sync.wait_ge(sAsc, jo + 1)
                        src3 = hb[jo % 4][:].rearrange("p (c d) -> p c d", d=D)
                        dst3 = bass.AP(t_hp, jo * 8 * 128 * D,
                                       [[D, 128], [128 * D, 8], [1, D]])
                        sync.dma_start(dst3, src3).then_inc(sAout[jo % 4], 16)
                # zero rows of slot-prefix tables + uncovered h' zero rows
                sync.wait_ge(sFin, 1)
                for w in range(NW):
                    sync.dma_start(t_sp[w, 0:1, :], zrow[:]).then_inc(sCz, 16)
                nz_hp = 0
                for w in range(NW):
                    zr = w * WSTRIDE + ZROW
                    if zr >= AIT_REAL * 1024:
                        sync.dma_start(t_hp[zr:zr + 1, :], zrow[:]
                                       ).then_inc(sCz, 16)
                        nz_hp += 1
                # phase B: idx loads interleaved with staging writes
                for g, (gw, gg) in enumerate(sched):
                    if gg == 0:
                        for i in range(4):
                            sync.wait_ge(sBg[i],
                                         ((Gsum[gw] + 3 - i) // 4) * 16)
                        gh = G1s[gw] // 2
                        if gh == 0:
                            sync.dma_start(
                                idx1s[:, :G1s[gw] * 128],
                                t_idx1[gw][:, :G1s[gw] * 128],
                            ).then_inc(sBidx, 32)
                        else:
                            sync.dma_start(
                                idx1s[:, :gh * 128],
                                t_idx1[gw][:, :gh * 128],
                            ).then_inc(sBidx, 16)
                            sync.wait_ge(sBidx, 32 * gw + 16)
                            sync.dma_start(
                                idx1s[:, gh * 128:G1s[gw] * 128],
                                t_idx1[gw][:, gh * 128:G1s[gw] * 128],
                            ).then_inc(sBidx, 16)
                    sync.wait_ge(sBcpV, g + 1)
                    sync.wait_ge(sBcpS, g + 1)
                    src3 = sps[g % 4][:].rearrange("s (j d) -> s j d", d=D)
                    dst3 = bass.AP(
                        t_sp,
                        (gw * SPROWS + 1 + gg * GCH * SPC) * D,
                        [[GCH * D, SPC], [D, GCH], [1, D]],
                    )
                    sync.dma_start(dst3, src3).then_inc(sBst[g % 4], 16)
                # final out
                sync.wait_ge(sCacc, 2 * NW + 1)
                out3 = bass.AP(t_out, 0, [[D, 128], [128 * D, NCOLS], [1, D]])
                sync.dma_start(out3, accs[:]).then_inc(sFin, 16)
                sync.wait_ge(sFin, 17)

            @block.tensor
            def _(tensor):
                tensor.wait_ge(sLD, 16 * 7)
                for it in range(AIT_REAL):
                    tensor.wait_ge(sAx[it % 4], (it // 4 + 1) * 16)
                    if it >= 8:
                        tensor.wait_ge(sAsc, it - 7)
                    for j in range(8):
                        ins = tensor.matmul(
                            psum[it % 8][:, j * D:(j + 1) * D],
                            xb[it % 4][:, j * 128:(j + 1) * 128],
                            WTs[:],
                            start=True, stop=True,
                        )
                    ins.then_inc(sAmm, 1)
                tensor.wait_ge(sAsc, AIT_REAL)
                for g, (gw, gg) in enumerate(sched):
                    tensor.wait_ge(sBg[g % 4], (g // 4 + 1) * 16)
                    if g >= 4:
                        tensor.wait_ge(sBcpV, g - 3)
                        tensor.wait_ge(sBcpS, g - 3)
                    rhs = msg[g % 4][:].rearrange("p c d -> p (c d)")
                    for half in range(2):
                        ins = tensor.matmul(
                            psum[(g % 4) * 2 + half][:SPC, :],
                            SPs[:],
                            rhs[:, half * 512:(half + 1) * 512],
                            start=True, stop=True,
                        )
                    ins.then_inc(sBmm, 1)

            @block.vector
            def _(vector):
                vector.wait_ge(sLD, 16 * 7)
                vector.reciprocal(dinvTs[:], degTs[:]).then_inc(sDin, 1)
                vector.reciprocal(dinvDs[:], degDs[:]).then_inc(sDin, 1)
                vector.wait_ge(sDin, 3)  # sqrt done on scalar
                for it in range(AIT_REAL):
                    vector.wait_ge(sAmm, it + 1)
                    if it >= 4:
                        vector.wait_ge(sAout[it % 4], ((it - 4) // 4 + 1) * 16)
                    ps3 = psum[it % 8][:].rearrange("p (c d) -> p c d", d=D)
                    hb3 = hb[it % 4][:].rearrange("p (c d) -> p c d", d=D)
                    dv = bcast(dinvTs[:, it * 8:(it + 1) * 8], D)
                    vector.tensor_tensor(
                        hb3, ps3, dv, op=mybir.AluOpType.mult
                    ).then_inc(sAsc, 1)
                g = 0
                cnt = 0
                for w in range(NW):
                    for gg in range(G1s[w]):
                        vector.wait_ge(sBmm, g + 1)
                        if g >= 4:
                            vector.wait_ge(sBst[g % 4],
                                           ((g - 4) // 4 + 1) * 16)
                        vector.tensor_copy(
                            sps[g % 4][:, :512], psum[(g % 4) * 2][:SPC, :]
                        ).then_inc(sBcpV, 1)
                        g += 1
                    vector.wait_ge(sCa, (w + 1) * L2K * 16)
                    if w == 0:
                        vector.wait_ge(sCb, L2K * 16)
                        vector.tensor_tensor(
                            accs[:], Ab[:], Bb[:], op=mybir.AluOpType.subtract
                        ).then_inc(sCacc, 1)
                        cnt += 1
                    else:
                        vector.wait_ge(sCacc, cnt)
                        vector.tensor_tensor(
                            accs[:], accs[:], Ab[:], op=mybir.AluOpType.add
                        ).then_inc(sCacc, 1)
                        cnt += 1
                        vector.wait_ge(sCb, (w + 1) * L2K * 16)
                        vector.wait_ge(sCacc, cnt)
                        vector.tensor_tensor(
                            accs[:], accs[:], Bb[:],
                            op=mybir.AluOpType.subtract,
                        ).then_inc(sCacc, 1)
                        cnt += 1
                dvD = bcast(dinvDs[:], D)
                vector.wait_ge(sCacc, cnt)
                vector.tensor_tensor(
                    accs[:], accs[:], dvD, op=mybir.AluOpType.mult
                ).then_inc(sCacc, 1)
                cnt += 1
                apb = bBCs[:]
                bb = bass.AP(apb.tensor, apb.offset,
                             [list(apb.ap[0]), [0, NCOLS], list(apb.ap[1])])
                vector.wait_ge(sCacc, cnt)
                vector.tensor_tensor(
                    accs[:], accs[:], bb, op=mybir.AluOpType.add
                ).then_inc(sCacc, 1)

            @block.scalar
            def _(scalar):
                scalar.wait_ge(sDin, 2)
                scalar.activation(dinvTs[:], dinvTs[:],
                                  mybir.ActivationFunctionType.Sqrt)
                scalar.activation(dinvDs[:], dinvDs[:],
                                  mybir.ActivationFunctionType.Sqrt
                                  ).then_inc(sDin, 1)
                for g in range(NG):
                    scalar.wait_ge(sBmm, g + 1)
                    if g >= 4:
                        scalar.wait_ge(sBst[g % 4], ((g - 4) // 4 + 1) * 16)
                    scalar.activation(
                        sps[g % 4][:, 512:1024],
                        psum[(g % 4) * 2 + 1][:SPC, :],
                        mybir.ActivationFunctionType.Copy,
                    ).then_inc(sBcpS, 1)

            @block.gpsimd
            def _(gpsimd: bass.BassGpSimd):
                gpsimd.load_library(mlp)
                gpsimd.memset(zrow[:], 0.0).then_inc(sFin, 1)
                AW = AIT // NW  # h' iters per window
                NZHP = sum(1 for w in range(NW)
                           if w * WSTRIDE + ZROW >= AIT_REAL * 1024)
                L2C = L2G // 128
                L2W = L2G // 16
                def phase_c(w):
                    # phase C for window w (after its staging lands)
                    for i in range(4):
                        gpsimd.wait_ge(sBst[i],
                                       ((Gsum[w + 1] + 3 - i) // 4) * 16)
                    if w == 0:
                        gpsimd.wait_ge(sCz, (NW + NZHP) * 16)
                    if w >= 1:
                        gpsimd.wait_ge(sCacc, 2 * w - 1)
                    for k in range(L2K):
                        gpsimd.dma_gather(
                            Ab[:, k * L2C:(k + 1) * L2C, :], t_sp[w],
                            idxas[:, w * NAB + k * L2W:
                                  w * NAB + (k + 1) * L2W],
                            L2G, L2G, D, single_packet=False,
                        ).then_inc(sCa, 16)
                    if w >= 1:
                        gpsimd.wait_ge(sCacc, 2 * w)
                    for k in range(L2K):
                        gpsimd.dma_gather(
                            Bb[:, k * L2C:(k + 1) * L2C, :], t_sp[w],
                            idxbs[:, w * NAB + k * L2W:
                                  w * NAB + (k + 1) * L2W],
                            L2G, L2G, D, single_packet=False,
                        ).then_inc(sCb, 16)

                g = 0
                for w in range(NW):
                    # h' rows of window w ready
                    up = min((w + 1) * AW, AIT_REAL)
                    for i in range(4):
                        gpsimd.wait_ge(sAout[i], ((up + 3 - i) // 4) * 16)
                    if up == AIT_REAL and NZHP:
                        gpsimd.wait_ge(sCz, (NW + NZHP) * 16)
                    gpsimd.wait_ge(sBidx, 32 * w + 16)
                    gh = G1s[w] // 2
                    for gg in range(G1s[w]):
                        if gg == gh and gh > 0:
                            gpsimd.wait_ge(sBidx, 32 * w + 32)
                        if g >= 4:
                            gpsimd.wait_ge(sBmm, g - 3)
                        hp_w = t_hp[w * WSTRIDE:(w + 1) * WSTRIDE, :]
                        gpsimd.dma_gather(
                            msg[g % 4][:], hp_w,
                            idx1s[:, gg * 128:(gg + 1) * 128],
                            NIDX, NIDX, D, single_packet=False,
                        ).then_inc(sBg[g % 4], 16)
                        g += 1
                    phase_c(w)

        nc.compile()
    return nc


def kernel(x, edge_index, W, b):
    x = np.asarray(x, dtype=np.float32)
    edge_index = np.asarray(edge_index)
    W = np.asarray(W, dtype=np.float32)
    b = np.asarray(b, dtype=np.float32)

    src = np.asarray(edge_index[0], dtype=np.int64)
    dst = np.asarray(edge_index[1], dtype=np.int64)

    deg = (np.bincount(dst, minlength=N) + 1.0).astype(np.float32)

    tbl_row = np.arange(N) + np.arange(N) // WCAP
    x_table = np.zeros((TBL, D), np.float32)
    x_table[tbl_row] = x
    xT = np.ascontiguousarray(x_table.T)
    deg_table = np.ones(TBL, np.float32)
    deg_table[tbl_row] = deg
    degT = np.ascontiguousarray(deg_table.reshape(TBL // 128, 128).T)

    WT = np.ascontiguousarray(W.T)
    SP = (np.arange(128)[:, None] <= (L * np.arange(SPC) + L - 1)[None, :]
          ).astype(np.float32)
    bBC = np.ascontiguousarray(np.broadcast_to(b, (128, D)))

    shard_of = dst // SHARD
    order_c = np.argsort(shard_of, kind="stable")
    bounds = np.searchsorted(shard_of[order_c], np.arange(NCORES + 1))
    cores = []
    for c in range(NCORES):
        sel = order_c[bounds[c]:bounds[c + 1]]
        es = np.concatenate([src[sel],
                             np.arange(c * SHARD, (c + 1) * SHARD)])
        ed = np.concatenate([dst[sel] - c * SHARD, np.arange(SHARD)])
        cores.append(_prep_core(es, ed))

    C1s = []
    for w in range(NW):
        cw = max(cr["C"][w] for cr in cores)
        C1s.append(max(GCH, -(-cw // GCH) * GCH))
    C1 = max(C1s)
    G1s = [c // GCH for c in C1s]
    SPROWS = 1 + C1 * SPC
    assert SPROWS <= 32767, f"slot-prefix table too large: {SPROWS}"

    NAB = OUTROWS // 16
    in_maps = []
    for c in range(NCORES):
        cr = cores[c]
        idx1 = np.empty((NW, 128, C1 * 8), np.int16)
        idxa = np.empty((128, NW * NAB), np.int16)
        idxb = np.empty((128, NW * NAB), np.int16)
        for w in range(NW):
            fl = np.full(C1 * 128, ZROW, np.int64)
            fl[:cr["idx"][w].shape[0]] = cr["idx"][w]
            parts = [_wrap16(fl[g * NIDX:(g + 1) * NIDX])
                     for g in range(C1 // GCH)]
            idx1[w] = np.concatenate(parts, axis=1)
            for arr, dest in ((cr["a"][w], idxa), (cr["b"][w], idxb)):
                af = np.zeros(OUTROWS, np.int64)
                af[:SHARD] = arr
                parts = [_wrap16(af[k * L2G:(k + 1) * L2G])
                         for k in range(L2K)]
                dest[:, w * NAB:(w + 1) * NAB] = np.concatenate(parts, axis=1)
        degD_flat = np.ones(OUTROWS, np.float32)
        degD_flat[:SHARD] = deg[c * SHARD:(c + 1) * SHARD]
        degD = np.ascontiguousarray(degD_flat.reshape(NCOLS, 128).T)
        in_maps.append({
            "xT": xT, "WT": WT, "SP": SP, "degT": degT, "degD": degD,
            "bBC": bBC, "idx1": idx1, "idxa": idxa, "idxb": idxb,
        })

    nc = _build_program(C1, SPROWS, G1s)
    global LAST_NC
    LAST_NC = nc
    res = bass_utils.run_bass_kernel_spmd(nc, in_maps,
                                          core_ids=list(range(NCORES)))
    out = np.empty((N, D), np.float32)
    for c in range(NCORES):
        out[c * SHARD:(c + 1) * SHARD] = res.results[c]["out_s"][:SHARD]
    return out



# revision 6
# speedup vs baseline: 1.6132x; 1.6132x over previous
"""GCNConv (PyG semantics) on 8 Trainium2 NeuronCores.

out = D^-1/2 (A+I) D^-1/2 (x @ W.T) + b, dst-sharded across 8 cores.

Host: bucket edges (plus self loops) by dst shard; split each node's edge
list by 4 source windows of 32,767 rows (dma_gather indices are int16); pad
each (node, window) run into L-edge slots; pack slots into 128-edge chunks
(SPC slots/chunk).

Device, per core (all message data in fp16; 128-byte gather descriptors):
  Phase A: h' = (x @ W.T) * dinv[row] over a 131072-row padded table
           (1 zero row per window); written to HBM as fp16 rows strided
           256B ([TBL, 128] with payload in cols 0:64).
  Phase B: per window, dma_gather 4096 messages at a time from h' (fp16,
           128B per descriptor); PE matmul with a constant slot-prefix
           matrix SP [128, SPC] -> PSUM (slot-prefix sums); DVE/ACT copy
           to SBUF fp16; DMA-stage to a DRAM slot-prefix table (row 0
           reserved zero, rows strided 256B).
  Phase C: per window, dma_gather per-node boundary prefix rows (a = last
           slot, b = before first slot) as fp16, acc += A_w - B_w in fp32;
           then scale by dinv[dst], add bias, write out shard fp32.
"""

import numpy as np
from contextlib import ExitStack

import concourse.bacc as bacc
import concourse.bass as bass
import concourse.mybir as mybir
from concourse import ap_utils, bass_utils
from concourse.bass import MemorySpace
from concourse.library_config import mlp

D = 64


def configure(n=100000, ncores=8, wcap=20479, nw=5, l2g=1792, slot=4,
              gch=32):
    # set problem geometry (module globals); defaults = real problem
    global N, NCORES, SHARD, WCAP, NW, WSTRIDE, TBL, ZROW
    global OUTROWS, NCOLS, L2G, L2K, AIT, AIT_REAL
    global L, SPC, GCH, NIDX
    N = n
    NCORES = ncores
    SHARD = N // NCORES
    WCAP = wcap
    NW = nw
    WSTRIDE = WCAP + 1
    assert WSTRIDE % 128 == 0 and NW * WCAP >= N
    TBL = NW * WSTRIDE
    ZROW = WCAP
    OUTROWS = -(-SHARD // 128) * 128
    NCOLS = OUTROWS // 128
    L2G = l2g
    assert OUTROWS % L2G == 0 and L2G % 128 == 0
    L2K = OUTROWS // L2G
    assert TBL % 1024 == 0
    AIT = TBL // 1024
    max_row = (N - 1) + (N - 1) // WCAP
    AIT_REAL = -(-(max_row + 1) // 1024)
    L = slot
    SPC = 128 // L
    GCH = gch
    NIDX = GCH * 128


configure()
LAST_NC = None


def dma_gather_raw(eng, out_ap, in_ap, idxs_ap, num_idxs, num_idxs_reg,
                   elem_size, elem_step, single_packet=False, queue_num=0):
    """bass.BassGpSimd.dma_gather minus the elem%256B assert (non-transpose
    HBM source; the ucode packetizes any elem size — only the table STRIDE
    must be a multiple of 256B). Verified on HW with 128B fp16 elems."""
    assert idxs_ap.dtype == mybir.dt.int16
    assert in_ap.dtype == out_ap.dtype
    assert in_ap.space == MemorySpace.DRAM
    assert idxs_ap.space == MemorySpace.SBUF
    assert out_ap.space == MemorySpace.SBUF
    assert ap_utils.ap_is_contiguous(in_ap.ap[1:])
    assert ap_utils.ap_is_contiguous(out_ap.ap[1:])
    assert ap_utils.ap_is_contiguous(idxs_ap.ap[1:])
    assert in_ap.ap[-1][1] == out_ap.ap[-1][1] == elem_size
    assert out_ap.ap[0][1] * out_ap.ap[1][1] == bass.round_up_to_multiple(
        num_idxs, 128)
    assert in_ap.ap[0][0] == elem_step
    stride_bytes = elem_step * mybir.dt.size(in_ap.dtype)
    stride_bytes_256 = bass.exact_div(stride_bytes, 256)
    assert stride_bytes_256 < 256
    return eng.add_instruction(
        mybir.InstDMAGatherAnt(
            name=eng.bass.get_next_instruction_name(),
            ins=[*eng.lower_ap_dma(in_ap, for_custom_bir_dma=True),
                 eng.lower_ap(idxs_ap),
                 eng.lower_val_access(eng.to_reg(num_idxs_reg))],
            outs=[eng.lower_ap(out_ap)],
            transpose=False,
            num_idxs=num_idxs,
            elem_size=elem_size,
            stride_bytes_256=stride_bytes_256,
            gen_mode=0,
            single_packet=single_packet,
            queue_num=queue_num,
            sbuf_tokens_per_rank=0,
            sbuf_free_dim_per_rank=0,
            sbuf_free_dim_pad_per_rank=0,
            sbuf_byte_offset=0,
        )
    )


def _wrap16(idx_flat):
    """Flat idx list -> dma_gather int16 wrap [16, n//16] tiled to 128."""
    n = idx_flat.shape[0]
    out = idx_flat.reshape(n // 16, 16).T.astype(np.int16)
    return np.tile(out, (8, 1))


def _prep_core(src_g, dst_l):
    """Per-core, per-window gather/aggregation structures."""
    win = src_g // WCAP
    loc = src_g - win * WCAP  # 0..32766

    res = {"idx": [], "a": [], "b": [], "C": []}
    for w in range(NW):
        m = win == w
        dw = dst_l[m]
        lw = loc[m]
        order = np.argsort(dw, kind="stable")
        dw = dw[order]
        lw = lw[order]
        counts = np.bincount(dw, minlength=SHARD)
        slots = -(-counts // L)
        assert (slots <= SPC).all(), "node needs more slots than SPC"
        # first-fit pack node slot-runs into SPC-slot chunks, in node order
        slot_start = np.zeros(SHARD, np.int64)
        chunk_of = np.zeros(SHARD, np.int64)
        cur_chunk, cur_slot = 0, 0
        nz_idx = np.nonzero(counts)[0]
        for n_ in nz_idx:
            s = slots[n_]
            if cur_slot + s > SPC:
                cur_chunk += 1
                cur_slot = 0
            chunk_of[n_] = cur_chunk
            slot_start[n_] = cur_slot
            cur_slot += s
        C = cur_chunk + (1 if cur_slot > 0 else 0)
        nz = counts > 0
        starts = np.zeros(SHARD, np.int64)
        starts[1:] = np.cumsum(counts)[:-1]
        pos_base = chunk_of * 128 + slot_start * L
        idx = np.full(max(C, 1) * 128, ZROW, np.int64)
        within = np.arange(dw.shape[0]) - np.repeat(starts[nz], counts[nz])
        pos = np.repeat(pos_base[nz], counts[nz]) + within
        idx[pos] = lw
        a = np.zeros(SHARD, np.int64)
        b = np.zeros(SHARD, np.int64)

        def sprow(ch, sl):
            return (ch // GCH) * (GCH * SPC) + sl * GCH + ch % GCH + 1

        a[nz] = sprow(chunk_of[nz], slot_start[nz] + slots[nz] - 1)
        sb0 = slot_start[nz] > 0
        bnz = np.zeros(int(nz.sum()), np.int64)
        bnz[sb0] = sprow(chunk_of[nz][sb0], slot_start[nz][sb0] - 1)
        b[nz] = bnz
        res["idx"].append(idx)
        res["a"].append(a)
        res["b"].append(b)
        res["C"].append(C)
    return res


def _build_program(C1, SPROWS, G1s):
    sched = [(w, gg) for w in range(NW) for gg in range(G1s[w])]
    NG = len(sched)             # total phase-B groups
    Gsum = [0]
    for w in range(NW):
        Gsum.append(Gsum[-1] + G1s[w])
    dt = mybir.dt
    TCOL = TBL // 128           # 1024
    NAB = OUTROWS // 16         # idx cols per (window, a|b)
    IGC = NIDX // 16            # idx cols per phase-B group

    nc = bacc.Bacc("TRN2", target_bir_lowering=False, debug=False,
                   num_devices=NCORES)
    t_xT = nc.dram_tensor("xT", [D, TBL], dt.float16, kind="ExternalInput")
    t_WT = nc.dram_tensor("WT", [D, D], dt.float16, kind="ExternalInput")
    t_SP = nc.dram_tensor("SP", [128, SPC], dt.float16,
                          kind="ExternalInput")
    t_degT = nc.dram_tensor("degT", [128, TCOL], dt.float32,
                            kind="ExternalInput")
    t_degD = nc.dram_tensor("degD", [128, NCOLS], dt.float32,
                            kind="ExternalInput")
    t_bBC = nc.dram_tensor("bBC", [128, D], dt.float32, kind="ExternalInput")
    t_idx1 = nc.dram_tensor("idx1", [NW, 128, C1 * 8], dt.int16,
                            kind="ExternalInput")
    t_idxa = nc.dram_tensor("idxa", [128, NW * NAB], dt.int16,
                            kind="ExternalInput")
    t_idxb = nc.dram_tensor("idxb", [128, NW * NAB], dt.int16,
                            kind="ExternalInput")
    t_hp = nc.dram_tensor("hp", [TBL, 128], dt.float16)
    t_sp = nc.dram_tensor("sp", [NW, SPROWS, 128], dt.float16)
    t_out = nc.dram_tensor("out_s", [OUTROWS, D], dt.float32,
                           kind="ExternalOutput")

    with ExitStack() as ctx:
        e = ctx.enter_context
        xb = [e(nc.sbuf_tensor(f"xb{i}", [D, 1024], dt.float16))
              for i in range(4)]
        hb = [e(nc.sbuf_tensor(f"hb{i}", [128, 512], dt.float16))
              for i in range(4)]
        WTs = e(nc.sbuf_tensor("WTs", [D, D], dt.float16))
        SPs = e(nc.sbuf_tensor("SPs", [128, SPC], dt.float16))
        bBCs = e(nc.sbuf_tensor("bBCs", [128, D], dt.float32))
        degTs = e(nc.sbuf_tensor("degTs", [128, TCOL], dt.float32))
        dinvTs = e(nc.sbuf_tensor("dinvTs", [128, TCOL], dt.float32))
        degDs = e(nc.sbuf_tensor("degDs", [128, NCOLS], dt.float32))
        dinvDs = e(nc.sbuf_tensor("dinvDs", [128, NCOLS], dt.float32))
        idx1s = e(nc.sbuf_tensor("idx1s", [128, C1 * 8], dt.int16))
        idxas = e(nc.sbuf_tensor("idxas", [128, NW * NAB], dt.int16))
        idxbs = e(nc.sbuf_tensor("idxbs", [128, NW * NAB], dt.int16))
        msg = [e(nc.sbuf_tensor(f"msg{i}", [128, GCH, D], dt.float16))
               for i in range(4)]
        sps = [e(nc.sbuf_tensor(f"sps{i}", [SPC, GCH * D], dt.float16))
               for i in range(4)]
        zrow = e(nc.sbuf_tensor("zrow", [1, D], dt.float16))
        Ab = e(nc.sbuf_tensor("Ab", [128, NCOLS, D], dt.float16))
        Bb = e(nc.sbuf_tensor("Bb", [128, NCOLS, D], dt.float16))
        accs = e(nc.sbuf_tensor("accs", [128, NCOLS, D], dt.float32))
        psum = [e(nc.psum_tensor(f"ps{i}", [128, 512], dt.float32))
                for i in range(8)]

        sLD = e(nc.semaphore("sLD"))
        sAx = [e(nc.semaphore(f"sAx{i}")) for i in range(4)]
        sAmm = e(nc.semaphore("sAmm"))
        sAsc = e(nc.semaphore("sAsc"))
        sAout = [e(nc.semaphore(f"sAout{i}")) for i in range(4)]
        sDin = e(nc.semaphore("sDin"))
        sBidx = e(nc.semaphore("sBidx"))
        sBg = [e(nc.semaphore(f"sBg{i}")) for i in range(4)]
        sBmm = e(nc.semaphore("sBmm"))
        sBcpV = e(nc.semaphore("sBcpV"))
        sBcpS = e(nc.semaphore("sBcpS"))
        sBst = [e(nc.semaphore(f"sBst{i}")) for i in range(4)]
        sCz = e(nc.semaphore("sCz"))
        sCa = e(nc.semaphore("sCa"))
        sCb = e(nc.semaphore("sCb"))
        sCacc = e(nc.semaphore("sCacc"))
        sFin = e(nc.semaphore("sFin"))

        def bcast(ap, reps):
            return bass.AP(ap.tensor, ap.offset, list(ap.ap) + [[0, reps]])

        with nc.Block() as block:

            @block.sync
            def _(sync: bass.BassEngine):
                sync.dma_start(WTs[:], t_WT[:]).then_inc(sLD, 16)
                sync.dma_start(SPs[:], t_SP[:]).then_inc(sLD, 16)
                sync.dma_start(bBCs[:], t_bBC[:]).then_inc(sLD, 16)
                sync.dma_start(degTs[:], t_degT[:]).then_inc(sLD, 16)
                sync.dma_start(degDs[:], t_degD[:]).then_inc(sLD, 16)
                sync.dma_start(idxas[:], t_idxa[:]).then_inc(sLD, 16)
                sync.dma_start(idxbs[:], t_idxb[:]).then_inc(sLD, 16)
                # phase A, interleaved x-in / h'-out
                for it in range(AIT_REAL + 2):
                    if it < AIT_REAL:
                        if it >= 4:
                            sync.wait_ge(sAmm, it - 3)
                        sync.dma_start(
                            xb[it % 4][:], t_xT[:, it * 1024:(it + 1) * 1024]
                        ).then_inc(sAx[it % 4], 16)
                    if it >= 2:
                        jo = it - 2
                        sync.wait_ge(sAsc, jo + 1)
                        src3 = hb[jo % 4][:].rearrange("p (c d) -> p c d",
                                                       d=D)
                        dst3 = bass.AP(t_hp, jo * 1024 * 128,
                                       [[128, 128], [128 * 128, 8], [1, D]])
                        sync.dma_start(dst3, src3).then_inc(sAout[jo % 4], 16)
                # zero rows of slot-prefix tables + uncovered h' zero rows
                sync.wait_ge(sFin, 1)
                for w in range(NW):
                    sync.dma_start(t_sp[w, 0:1, 0:D], zrow[:]
                                   ).then_inc(sCz, 16)
                for w in range(NW):
                    zr = w * WSTRIDE + ZROW
                    if zr >= AIT_REAL * 1024:
                        sync.dma_start(t_hp[zr:zr + 1, 0:D], zrow[:]
                                       ).then_inc(sCz, 16)
                # phase B: idx loads interleaved with staging writes
                for g, (gw, gg) in enumerate(sched):
                    if gg == 0:
                        for i in range(4):
                            sync.wait_ge(sBg[i],
                                         ((Gsum[gw] + 3 - i) // 4) * 16)
                        gh = G1s[gw] // 2
                        if gh == 0:
                            sync.dma_start(
                                idx1s[:, :G1s[gw] * IGC],
                                t_idx1[gw][:, :G1s[gw] * IGC],
                            ).then_inc(sBidx, 32)
                        else:
                            sync.dma_start(
                                idx1s[:, :gh * IGC],
                                t_idx1[gw][:, :gh * IGC],
                            ).then_inc(sBidx, 16)
                            sync.wait_ge(sBidx, 32 * gw + 16)
                            sync.dma_start(
                                idx1s[:, gh * IGC:G1s[gw] * IGC],
                                t_idx1[gw][:, gh * IGC:G1s[gw] * IGC],
                            ).then_inc(sBidx, 16)
                    sync.wait_ge(sBcpV, g + 1)
                    sync.wait_ge(sBcpS, g + 1)
                    src3 = sps[g % 4][:].rearrange("s (j d) -> s j d", d=D)
                    dst3 = bass.AP(
                        t_sp,
                        (gw * SPROWS + 1 + gg * GCH * SPC) * 128,
                        [[GCH * 128, SPC], [128, GCH], [1, D]],
                    )
                    sync.dma_start(dst3, src3).then_inc(sBst[g % 4], 16)
                # final out
                sync.wait_ge(sCacc, 2 * NW + 1)
                out3 = bass.AP(t_out, 0, [[D, 128], [128 * D, NCOLS], [1, D]])
                sync.dma_start(out3, accs[:]).then_inc(sFin, 16)
                sync.wait_ge(sFin, 17)

            @block.tensor
            def _(tensor):
                tensor.wait_ge(sLD, 16 * 7)
                for it in range(AIT_REAL):
                    tensor.wait_ge(sAx[it % 4], (it // 4 + 1) * 16)
                    if it >= 8:
                        tensor.wait_ge(sAsc, it - 7)
                    for j in range(8):
                        ins = tensor.matmul(
                            psum[it % 8][:, j * D:(j + 1) * D],
                            xb[it % 4][:, j * 128:(j + 1) * 128],
                            WTs[:],
                            start=True, stop=True,
                        )
                    ins.then_inc(sAmm, 1)
                tensor.wait_ge(sAsc, AIT_REAL)
                for g, (gw, gg) in enumerate(sched):
                    tensor.wait_ge(sBg[g % 4], (g // 4 + 1) * 16)
                    if g >= 2:
                        tensor.wait_ge(sBcpV, g - 1)
                        tensor.wait_ge(sBcpS, g - 1)
                    rhs = msg[g % 4][:].rearrange("p c d -> p (c d)")
                    for q in range(4):
                        ins = tensor.matmul(
                            psum[(g % 2) * 4 + q][:SPC, :],
                            SPs[:],
                            rhs[:, q * 512:(q + 1) * 512],
                            start=True, stop=True,
                        )
                    ins.then_inc(sBmm, 1)

            @block.vector
            def _(vector):
                vector.wait_ge(sLD, 16 * 7)
                vector.reciprocal(dinvTs[:], degTs[:]).then_inc(sDin, 1)
                vector.reciprocal(dinvDs[:], degDs[:]).then_inc(sDin, 1)
                vector.wait_ge(sDin, 3)  # sqrt done on scalar
                for it in range(AIT_REAL):
                    vector.wait_ge(sAmm, it + 1)
                    if it >= 4:
                        vector.wait_ge(sAout[it % 4], ((it - 4) // 4 + 1) * 16)
                    ps3 = psum[it % 8][:].rearrange("p (c d) -> p c d", d=D)
                    hb3 = hb[it % 4][:].rearrange("p (c d) -> p c d", d=D)
                    dv = bcast(dinvTs[:, it * 8:(it + 1) * 8], D)
                    vector.tensor_tensor(
                        hb3, ps3, dv, op=mybir.AluOpType.mult
                    ).then_inc(sAsc, 1)
                g = 0
                cnt = 0
                for w in range(NW):
                    for gg in range(G1s[w]):
                        vector.wait_ge(sBmm, g + 1)
                        if g >= 4:
                            vector.wait_ge(sBst[g % 4],
                                           ((g - 4) // 4 + 1) * 16)
                        vector.tensor_copy(
                            sps[g % 4][:, 0:512],
                            psum[(g % 2) * 4][:SPC, :],
                        )
                        vector.tensor_copy(
                            sps[g % 4][:, 512:1024],
                            psum[(g % 2) * 4 + 1][:SPC, :],
                        ).then_inc(sBcpV, 1)
                        g += 1
                    vector.wait_ge(sCa, (w + 1) * L2K * 16)
                    if w == 0:
                        vector.wait_ge(sCb, L2K * 16)
                        vector.tensor_tensor(
                            accs[:], Ab[:], Bb[:], op=mybir.AluOpType.subtract
                        ).then_inc(sCacc, 1)
                        cnt += 1
                    else:
                        vector.wait_ge(sCacc, cnt)
                        vector.tensor_tensor(
                            accs[:], accs[:], Ab[:], op=mybir.AluOpType.add
                        ).then_inc(sCacc, 1)
                        cnt += 1
                        vector.wait_ge(sCb, (w + 1) * L2K * 16)
                        vector.wait_ge(sCacc, cnt)
                        vector.tensor_tensor(
                            accs[:], accs[:], Bb[:],
                            op=mybir.AluOpType.subtract,
                        ).then_inc(sCacc, 1)
                        cnt += 1
                dvD = bcast(dinvDs[:], D)
                vector.wait_ge(sCacc, cnt)
                vector.tensor_tensor(
                    accs[:], accs[:], dvD, op=mybir.AluOpType.mult
                ).then_inc(sCacc, 1)
                cnt += 1
                apb = bBCs[:]
                bb = bass.AP(apb.tensor, apb.offset,
                             [list(apb.ap[0]), [0, NCOLS], list(apb.ap[1])])
                vector.wait_ge(sCacc, cnt)
                vector.tensor_tensor(
                    accs[:], accs[:], bb, op=mybir.AluOpType.add
                ).then_inc(sCacc, 1)

            @block.scalar
            def _(scalar):
                scalar.wait_ge(sDin, 2)
                scalar.activation(dinvTs[:], dinvTs[:],
                                  mybir.ActivationFunctionType.Sqrt)
                scalar.activation(dinvDs[:], dinvDs[:],
                                  mybir.ActivationFunctionType.Sqrt
                                  ).then_inc(sDin, 1)
                for g in range(NG):
                    scalar.wait_ge(sBmm, g + 1)
                    if g >= 4:
                        scalar.wait_ge(sBst[g % 4], ((g - 4) // 4 + 1) * 16)
                    scalar.activation(
                        sps[g % 4][:, 1024:1536],
                        psum[(g % 2) * 4 + 2][:SPC, :],
                        mybir.ActivationFunctionType.Copy,
                    )
                    scalar.activation(
                        sps[g % 4][:, 1536:2048],
                        psum[(g % 2) * 4 + 3][:SPC, :],
                        mybir.ActivationFunctionType.Copy,
                    ).then_inc(sBcpS, 1)

            @block.gpsimd
            def _(gpsimd: bass.BassGpSimd):
                gpsimd.load_library(mlp)
                gpsimd.memset(zrow[:], 0.0).then_inc(sFin, 1)
                AW = AIT // NW  # h' iters per window
                NZHP = sum(1 for w in range(NW)
                           if w * WSTRIDE + ZROW >= AIT_REAL * 1024)
                L2C = L2G // 128
                L2W = L2G // 16

                def phase_c(w):
                    # phase C for window w (after its staging lands)
                    for i in range(4):
                        gpsimd.wait_ge(sBst[i],
                                       ((Gsum[w + 1] + 3 - i) // 4) * 16)
                    if w == 0:
                        gpsimd.wait_ge(sCz, (NW + NZHP) * 16)
                    if w >= 1:
                        gpsimd.wait_ge(sCacc, 2 * w - 1)
                    spw = bass.AP(t_sp, w * SPROWS * 128,
                                  [[128, SPROWS], [1, D]])
                    for k in range(L2K):
                        dma_gather_raw(
                            gpsimd,
                            Ab[:, k * L2C:(k + 1) * L2C, :], spw,
                            idxas[:, w * NAB + k * L2W:
                                  w * NAB + (k + 1) * L2W],
                            L2G, L2G, D, 128, single_packet=False,
                        ).then_inc(sCa, 16)
                    if w >= 1:
                        gpsimd.wait_ge(sCacc, 2 * w)
                    for k in range(L2K):
                        dma_gather_raw(
                            gpsimd,
                            Bb[:, k * L2C:(k + 1) * L2C, :], spw,
                            idxbs[:, w * NAB + k * L2W:
                                  w * NAB + (k + 1) * L2W],
                            L2G, L2G, D, 128, single_packet=False,
                        ).then_inc(sCb, 16)

                g = 0
                for w in range(NW):
                    # h' rows of window w ready
                    up = min((w + 1) * AW, AIT_REAL)
                    for i in range(4):
                        gpsimd.wait_ge(sAout[i], ((up + 3 - i) // 4) * 16)
                    if up == AIT_REAL and NZHP:
                        gpsimd.wait_ge(sCz, (NW + NZHP) * 16)
                    gpsimd.wait_ge(sBidx, 32 * w + 16)
                    gh = G1s[w] // 2
                    hp_w = bass.AP(t_hp, w * WSTRIDE * 128,
                                   [[128, WSTRIDE], [1, D]])
                    for gg in range(G1s[w]):
                        if gg == gh and gh > 0:
                            gpsimd.wait_ge(sBidx, 32 * w + 32)
                        if g >= 4:
                            gpsimd.wait_ge(sBmm, g - 3)
                        dma_gather_raw(
                            gpsimd,
                            msg[g % 4][:], hp_w,
                            idx1s[:, gg * IGC:(gg + 1) * IGC],
                            NIDX, NIDX, D, 128, single_packet=False,
                        ).then_inc(sBg[g % 4], 16)
                        g += 1
                    phase_c(w)

        nc.compile()
    return nc


def kernel(x, edge_index, W, b):
    x = np.asarray(x, dtype=np.float32)
    edge_index = np.asarray(edge_index)
    W = np.asarray(W, dtype=np.float32)
    b = np.asarray(b, dtype=np.float32)

    src = np.asarray(edge_index[0], dtype=np.int64)
    dst = np.asarray(edge_index[1], dtype=np.int64)

    deg = (np.bincount(dst, minlength=N) + 1.0).astype(np.float32)

    tbl_row = np.arange(N) + np.arange(N) // WCAP
    x_table = np.zeros((TBL, D), np.float16)
    x_table[tbl_row] = x.astype(np.float16)
    xT = np.ascontiguousarray(x_table.T)
    deg_table = np.ones(TBL, np.float32)
    deg_table[tbl_row] = deg
    degT = np.ascontiguousarray(deg_table.reshape(TBL // 128, 128).T)

    WT = np.ascontiguousarray(W.T.astype(np.float16))
    SP = (np.arange(128)[:, None] <= (L * np.arange(SPC) + L - 1)[None, :]
          ).astype(np.float16)
    bBC = np.ascontiguousarray(np.broadcast_to(b, (128, D))).astype(
        np.float32)

    shard_of = dst // SHARD
    order_c = np.argsort(shard_of, kind="stable")
    bounds = np.searchsorted(shard_of[order_c], np.arange(NCORES + 1))
    for slot_len in (L, 8):
        if slot_len != L:
            configure(n=N, ncores=NCORES, wcap=WCAP, nw=NW, l2g=L2G,
                      slot=slot_len, gch=GCH)
        cores = []
        for c in range(NCORES):
            sel = order_c[bounds[c]:bounds[c + 1]]
            es = np.concatenate([src[sel],
                                 np.arange(c * SHARD, (c + 1) * SHARD)])
            ed = np.concatenate([dst[sel] - c * SHARD, np.arange(SHARD)])
            cores.append(_prep_core(es, ed))
        C1s = []
        for w in range(NW):
            cw = max(cr["C"][w] for cr in cores)
            C1s.append(max(GCH, -(-cw // GCH) * GCH))
        C1 = max(C1s)
        G1s = [c // GCH for c in C1s]
        SPROWS = 1 + C1 * SPC
        if SPROWS <= 32767:
            break
    assert SPROWS <= 32767, f"slot-prefix table too large: {SPROWS}"

    NAB = OUTROWS // 16
    in_maps = []
    for c in range(NCORES):
        cr = cores[c]
        idx1 = np.empty((NW, 128, C1 * 8), np.int16)
        idxa = np.empty((128, NW * NAB), np.int16)
        idxb = np.empty((128, NW * NAB), np.int16)
        for w in range(NW):
            fl = np.full(C1 * 128, ZROW, np.int64)
            fl[:cr["idx"][w].shape[0]] = cr["idx"][w]
            parts = [_wrap16(fl[g * NIDX:(g + 1) * NIDX])
                     for g in range(C1 // GCH)]
            idx1[w] = np.concatenate(parts, axis=1)
            for arr, dest in ((cr["a"][w], idxa), (cr["b"][w], idxb)):
                af = np.zeros(OUTROWS, np.int64)
                af[:SHARD] = arr
                parts = [_wrap16(af[k * L2G:(k + 1) * L2G])
                         for k in range(L2K)]
                dest[:, w * NAB:(w + 1) * NAB] = np.concatenate(parts, axis=1)
        degD_flat = np.ones(OUTROWS, np.float32)
        degD_flat[:SHARD] = deg[c * SHARD:(c + 1) * SHARD]
        degD = np.ascontiguousarray(degD_flat.reshape(NCOLS, 128).T)
        in_maps.append({
            "xT": xT, "WT": WT, "SP": SP, "degT": degT, "degD": degD,
            "bBC": bBC, "idx1": idx1, "idxa": idxa, "idxb": idxb,
        })

    nc = _build_program(C1, SPROWS, G1s)
    global LAST_NC
    LAST_NC = nc
    res = bass_utils.run_bass_kernel_spmd(nc, in_maps,
                                          core_ids=list(range(NCORES)))
    out = np.empty((N, D), np.float32)
    for c in range(NCORES):
        out[c * SHARD:(c + 1) * SHARD] = res.results[c]["out_s"][:SHARD]
    return out


# revision 35
# speedup vs baseline: 2.0379x; 1.2632x over previous
"""GCNConv (PyG semantics) on 8 Trainium2 NeuronCores.

out = D^-1/2 (A+I) D^-1/2 (x @ W.T) + b, dst-sharded across 8 cores.

Host: bucket edges by dst shard; split each node's edge list by 4 equal
source windows of 25,087 rows (dma_gather indices are int16); pad each
(node, window) run into slots of L=4 edges; best-fit-decreasing pack slot
runs into 128-edge chunks. Self-loops are NOT edges: they are gathered
directly from the h' table by 4 per-window "self" gathers.

Device, per core (all message data fp16; 128-byte gather descriptors):
  Phase A: h' = (x @ W.T) * dinv[row] over a 100352-row padded table
           (1 zero row per window); written to HBM as fp16 rows strided
           256B ([TBL, 128], payload cols 0:64).
  Phase B: per window, dma_gather 4096 messages at a time from h'; PE
           matmul with the slot-prefix matrix SP [128, 32] -> PSUM
           (slot-prefix sums); DVE/ACT copy to SBUF fp16; DMA-stage to a
           per-window DRAM slot-prefix table (row 0 reserved zero).
  Phase C: per window, one dma_gather per boundary side (a = last slot,
           b = before first slot) into double-buffered fp16 tiles, plus a
           self gather from h'; acc += A_w - B_w + S_w (fp16); then scale
           by dinv[dst], add bias, write out shard fp16 (host -> fp32).
"""

import numpy as np
from contextlib import ExitStack

import concourse.bacc as bacc
import concourse.bass as bass
import concourse.mybir as mybir
from concourse import ap_utils, bass_utils
from concourse.bass import MemorySpace
from concourse.library_config import mlp

D = 64


def configure(n=100000, ncores=8, wcap=25087, nw=4, slot=4, gch=32):
    # set problem geometry (module globals); defaults = real problem
    global N, NCORES, SHARD, WCAP, NW, WSTRIDE, TBL, ZROW
    global OUTROWS, NCOLS, AIT, AIT_REAL
    global L, SPC, GCH, NIDX, CLIM
    N = n
    NCORES = ncores
    SHARD = N // NCORES
    WCAP = wcap
    NW = nw
    WSTRIDE = WCAP + 1
    assert WSTRIDE % 128 == 0 and NW * WCAP >= N
    TBL = NW * WSTRIDE
    ZROW = WCAP
    OUTROWS = -(-SHARD // 128) * 128
    NCOLS = OUTROWS // 128
    assert TBL % 1024 == 0
    AIT = TBL // 1024
    max_row = (N - 1) + (N - 1) // WCAP
    AIT_REAL = -(-(max_row + 1) // 1024)
    L = slot
    SPC = 128 // L         # slot-prefix columns per chunk
    GCH = gch
    NIDX = GCH * 128
    # per-window staging table must stay int16-addressable
    CLIM = (32767 - 1) // SPC


configure()
LAST_NC = None


def dma_gather_raw(eng, out_ap, in_ap, idxs_ap, num_idxs, num_idxs_reg,
                   elem_size, elem_step, single_packet=False, queue_num=0):
    """bass.BassGpSimd.dma_gather minus the elem%256B assert (non-transpose
    HBM source; the ucode packetizes any elem size — only the table STRIDE
    must be a multiple of 256B). Verified on HW with 128B fp16 elems."""
    assert idxs_ap.dtype == mybir.dt.int16
    assert in_ap.dtype == out_ap.dtype
    assert in_ap.space == MemorySpace.DRAM
    assert idxs_ap.space == MemorySpace.SBUF
    assert out_ap.space == MemorySpace.SBUF
    assert ap_utils.ap_is_contiguous(in_ap.ap[1:])
    assert ap_utils.ap_is_contiguous(out_ap.ap[1:])
    assert ap_utils.ap_is_contiguous(idxs_ap.ap[1:])
    assert in_ap.ap[-1][1] == out_ap.ap[-1][1] == elem_size
    assert out_ap.ap[0][1] * out_ap.ap[1][1] == bass.round_up_to_multiple(
        num_idxs, 128)
    assert in_ap.ap[0][0] == elem_step
    stride_bytes = elem_step * mybir.dt.size(in_ap.dtype)
    stride_bytes_256 = bass.exact_div(stride_bytes, 256)
    assert stride_bytes_256 < 256
    return eng.add_instruction(
        mybir.InstDMAGatherAnt(
            name=eng.bass.get_next_instruction_name(),
            ins=[*eng.lower_ap_dma(in_ap, for_custom_bir_dma=True),
                 eng.lower_ap(idxs_ap),
                 eng.lower_val_access(eng.to_reg(num_idxs_reg))],
            outs=[eng.lower_ap(out_ap)],
            transpose=False,
            num_idxs=num_idxs,
            elem_size=elem_size,
            stride_bytes_256=stride_bytes_256,
            gen_mode=0,
            single_packet=single_packet,
            queue_num=queue_num,
            sbuf_tokens_per_rank=0,
            sbuf_free_dim_per_rank=0,
            sbuf_free_dim_pad_per_rank=0,
            sbuf_byte_offset=0,
        )
    )


def _wrap16(idx_flat):
    """Flat idx list -> dma_gather int16 wrap [16, n//16] tiled to 64."""
    n = idx_flat.shape[0]
    out = idx_flat.reshape(n // 16, 16).T.astype(np.int16)
    return np.tile(out, (4, 1))


def _pack_cell(counts, Lc):
    """Best-fit-ish decreasing pack of ceil(counts/Lc) slot runs into
    128-edge chunks (capacity 128//Lc slots)."""
    cap = 128 // Lc
    nz = np.nonzero(counts)[0]
    sizes = -(-counts[nz] // Lc)
    assert (sizes <= cap).all()
    smax = int(sizes.max()) if nz.size else 1
    by_size = [np.nonzero(sizes == s)[0] for s in range(smax + 1)]
    ptr = [0] * (smax + 1)
    remaining = int(nz.size)
    chunk_of = np.zeros(nz.size, np.int64)
    slot_start = np.zeros(nz.size, np.int64)
    cur = 0
    while remaining:
        space = cap
        while True:
            s = min(space, smax)
            while s > 0 and ptr[s] >= len(by_size[s]):
                s -= 1
            if s == 0:
                break
            k = by_size[s][ptr[s]]
            ptr[s] += 1
            chunk_of[k] = cur
            slot_start[k] = cap - space
            space -= s
            remaining -= 1
        cur += 1
    return nz, sizes, chunk_of, slot_start, cur


def _prep_core(src_g, dst_l):
    """Per-core, per-window gather/aggregation structures."""
    win = src_g // WCAP
    loc = src_g - win * WCAP  # 0..WCAP-1

    res = {"idx": [], "a": [], "b": [], "C": []}
    for w in range(NW):
        m = win == w
        dw = dst_l[m]
        lw = loc[m]
        order = np.argsort(dw, kind="stable")
        dw = dw[order]
        lw = lw[order]
        counts = np.bincount(dw, minlength=SHARD)
        nz, sizes, chunk_of, slot_start, C = _pack_cell(counts, L)
        assert C <= CLIM, f"cell needs {C} chunks > {CLIM}"
        starts = np.zeros(SHARD, np.int64)
        starts[1:] = np.cumsum(counts)[:-1]
        pos_base = chunk_of * 128 + slot_start * L
        idx = np.full(max(C, 1) * 128, ZROW, np.int64)
        within = np.arange(dw.shape[0]) - np.repeat(starts[nz], counts[nz])
        pos = np.repeat(pos_base, counts[nz]) + within
        idx[pos] = lw

        def sprow(ch, col):
            return (ch // GCH) * (GCH * SPC) + col * GCH + ch % GCH + 1

        a = np.zeros(SHARD, np.int64)
        b = np.zeros(SHARD, np.int64)
        a[nz] = sprow(chunk_of, slot_start + sizes - 1)
        sb0 = slot_start > 0
        bnz = np.zeros(nz.size, np.int64)
        bnz[sb0] = sprow(chunk_of[sb0], slot_start[sb0] - 1)
        b[nz] = bnz
        res["idx"].append(idx)
        res["a"].append(a)
        res["b"].append(b)
        res["C"].append(C)
    return res


def _build_program(C1, SPR, G1s):
    sched = [(w, gg) for w in range(NW) for gg in range(G1s[w])]
    NG = len(sched)             # total phase-B groups
    Gsum = [0]
    for w in range(NW):
        Gsum.append(Gsum[-1] + G1s[w])
    SPOFF = [0]                 # per-window staging table row offsets
    for w in range(NW):
        SPOFF.append(SPOFF[-1] + SPR[w])
    dt = mybir.dt
    TCOL = TBL // 128
    NAB = OUTROWS // 16         # idx cols per (window, a|b|s)
    IGC = NIDX // 16            # idx cols per phase-B group
    IWC = C1 * 8                # idx cols per window
    # h'-write iterations covering window w
    UPW = [min(-(-((w + 1) * WSTRIDE) // 1024), AIT_REAL) for w in range(NW)]
    # static interleave of phase-A iterations and phase-B groups on PE/DVE:
    # B-group g becomes eligible ~2 A-iters after its gather could start.
    ELIG = [min(UPW[gw] + 2 * (g - Gsum[gw]) + 2, AIT_REAL)
            for g, (gw, gg) in enumerate(sched)]
    MERGED = []
    _gq = 0
    for _it in range(AIT_REAL):
        MERGED.append(("A", _it))
        while _gq < len(sched) and ELIG[_gq] <= _it + 1:
            MERGED.append(("B", _gq))
            _gq += 1
    while _gq < len(sched):
        MERGED.append(("B", _gq))
        _gq += 1
    # consumed-op counters: per window ops (+A/-B or A-B), +S
    cnt_a = [1, 3, 6, 9][:NW]       # op index when A_w consumed
    cnt_b = [1, 4, 7, 10][:NW]      # op index when B_w consumed
    cnt_s = [2, 5, 8, 11][:NW]      # op index when S_w consumed

    nc = bacc.Bacc("TRN2", target_bir_lowering=False, debug=False,
                   num_devices=NCORES)
    t_xT = nc.dram_tensor("xT", [D, TBL], dt.float16, kind="ExternalInput")
    t_WT = nc.dram_tensor("WT", [D, D], dt.float16, kind="ExternalInput")
    t_SP = nc.dram_tensor("SP", [128, SPC], dt.float16,
                          kind="ExternalInput")
    t_degT = nc.dram_tensor("degT", [128, TCOL], dt.float32,
                            kind="ExternalInput")
    t_degD = nc.dram_tensor("degD", [128, NCOLS], dt.float32,
                            kind="ExternalInput")
    t_bBC = nc.dram_tensor("bBC", [128, D], dt.float32, kind="ExternalInput")
    t_idx1 = nc.dram_tensor("idx1", [NW, 64, IWC], dt.int16,
                            kind="ExternalInput")
    t_idxa = nc.dram_tensor("idxa", [64, NW * NAB], dt.int16,
                            kind="ExternalInput")
    t_idxb = nc.dram_tensor("idxb", [64, NW * NAB], dt.int16,
                            kind="ExternalInput")
    t_idxs = nc.dram_tensor("idxs", [64, NW * NAB], dt.int16,
                            kind="ExternalInput")
    t_hp = nc.dram_tensor("hp", [TBL, 128], dt.float16)
    t_sp = nc.dram_tensor("sp", [SPOFF[NW], 128], dt.float16)
    t_out = nc.dram_tensor("out_s", [OUTROWS, D], dt.float16,
                           kind="ExternalOutput")

    with ExitStack() as ctx:
        e = ctx.enter_context
        xb = [e(nc.sbuf_tensor(f"xb{i}", [D, 1024], dt.float16))
              for i in range(4)]
        hb = [e(nc.sbuf_tensor(f"hb{i}", [128, 512], dt.float16))
              for i in range(4)]
        WTs = e(nc.sbuf_tensor("WTs", [D, D], dt.float16))
        SPs = e(nc.sbuf_tensor("SPs", [128, SPC], dt.float16))
        bBCs = e(nc.sbuf_tensor("bBCs", [128, D], dt.float32))
        degTs = e(nc.sbuf_tensor("degTs", [128, TCOL], dt.float32))
        dinvTs = e(nc.sbuf_tensor("dinvTs", [128, TCOL], dt.float32))
        degDs = e(nc.sbuf_tensor("degDs", [128, NCOLS], dt.float32))
        dinvDs = e(nc.sbuf_tensor("dinvDs", [128, NCOLS], dt.float32))
        idx1s = e(nc.sbuf_tensor("idx1s", [64, 2 * IWC], dt.int16))
        idxas = e(nc.sbuf_tensor("idxas", [64, NW * NAB], dt.int16))
        idxbs = e(nc.sbuf_tensor("idxbs", [64, NW * NAB], dt.int16))
        idxss = e(nc.sbuf_tensor("idxss", [64, NW * NAB], dt.int16))
        msg = [e(nc.sbuf_tensor(f"msg{i}", [128, GCH, D], dt.float16))
               for i in range(4)]
        sps = [e(nc.sbuf_tensor(f"sps{i}", [SPC, GCH * D], dt.float16))
               for i in range(4)]
        zrow = e(nc.sbuf_tensor("zrow", [1, D], dt.float16))
        Ab = [e(nc.sbuf_tensor(f"Ab{i}", [128, NCOLS, D], dt.float16))
              for i in range(2)]
        Bb = [e(nc.sbuf_tensor(f"Bb{i}", [128, NCOLS, D], dt.float16))
              for i in range(2)]
        Sb = e(nc.sbuf_tensor("Sb", [128, NCOLS, D], dt.float16))
        accs = e(nc.sbuf_tensor("accs", [128, NCOLS, D], dt.float16))
        psum = [e(nc.psum_tensor(f"ps{i}", [128, 512], dt.float32))
                for i in range(8)]

        sW = e(nc.semaphore("sW"))
        sDeg = e(nc.semaphore("sDeg"))
        sLD3 = e(nc.semaphore("sLD3"))
        sAx = [e(nc.semaphore(f"sAx{i}")) for i in range(4)]
        sAmm = e(nc.semaphore("sAmm"))
        sAsc = e(nc.semaphore("sAsc"))
        sAout = [e(nc.semaphore(f"sAout{i}")) for i in range(4)]
        sDin = e(nc.semaphore("sDin"))
        sBidx = [e(nc.semaphore(f"sBidx{i}")) for i in range(2)]
        sBg = [e(nc.semaphore(f"sBg{i}")) for i in range(4)]
        sBmm = e(nc.semaphore("sBmm"))
        sBcpV = e(nc.semaphore("sBcpV"))
        sBcpS = e(nc.semaphore("sBcpS"))
        sBst = [e(nc.semaphore(f"sBst{i}")) for i in range(4)]
        sCz = e(nc.semaphore("sCz"))
        sCa = e(nc.semaphore("sCa"))
        sCb = e(nc.semaphore("sCb"))
        sCs = e(nc.semaphore("sCs"))
        sCacc = e(nc.semaphore("sCacc"))
        sFin = e(nc.semaphore("sFin"))

        def bcast(ap, reps):
            return bass.AP(ap.tensor, ap.offset, list(ap.ap) + [[0, reps]])

        with nc.Block() as block:

            @block.sync
            def _(sync: bass.BassEngine):
                # x tile 0 first so phase A starts immediately
                sync.dma_start(xb[0][:], t_xT[:, 0:1024]).then_inc(sAx[0], 16)
                sync.dma_start(WTs[:], t_WT[:]).then_inc(sW, 16)
                sync.dma_start(SPs[:], t_SP[:]).then_inc(sW, 16)
                sync.dma_start(degTs[:], t_degT[:]).then_inc(sDeg, 16)
                sync.dma_start(degDs[:], t_degD[:]).then_inc(sDeg, 16)
                sync.dma_start(idx1s[:, 0:G1s[0] * IGC],
                               t_idx1[0][:, :G1s[0] * IGC]
                               ).then_inc(sBidx[0], 32)
                sync.dma_start(idx1s[:, IWC:IWC + G1s[1] * IGC],
                               t_idx1[1][:, :G1s[1] * IGC]
                               ).then_inc(sBidx[1], 32)
                # phase A, interleaved x-in / h'-out
                for it in range(AIT_REAL + 2):
                    if 0 < it < AIT_REAL:
                        if it >= 4:
                            sync.wait_ge(sAmm, it - 3)
                        sync.dma_start(
                            xb[it % 4][:], t_xT[:, it * 1024:(it + 1) * 1024]
                        ).then_inc(sAx[it % 4], 16)
                    if it >= 2:
                        jo = it - 2
                        sync.wait_ge(sAsc, jo + 1)
                        src3 = hb[jo % 4][:].rearrange("p (c d) -> p c d",
                                                       d=D)
                        dst3 = bass.AP(t_hp, jo * 1024 * 128,
                                       [[128, 128], [128 * 128, 8], [1, D]])
                        sync.dma_start(dst3, src3).then_inc(sAout[jo % 4], 16)
                # final out (two column halves to shorten the tail)
                NH = NCOLS // 2
                sync.wait_ge(sCacc, 3 * NW - 4 + 5)
                o3a = bass.AP(t_out, 0, [[D, 128], [128 * D, NH], [1, D]])
                sync.dma_start(o3a, accs[:, 0:NH, :]).then_inc(sFin, 16)
                sync.wait_ge(sCacc, 3 * NW - 4 + 10)
                o3b = bass.AP(t_out, NH * 128 * D,
                              [[D, 128], [128 * D, NCOLS - NH], [1, D]])
                sync.dma_start(o3b, accs[:, NH:NCOLS, :]).then_inc(sFin, 16)
                sync.wait_ge(sFin, 33)

            @block.tensor
            def _(tensor):
                tensor.wait_ge(sW, 16)
                for kind, i in MERGED:
                    if kind == "A":
                        it = i
                        tensor.wait_ge(sAx[it % 4], (it // 4 + 1) * 16)
                        if it >= 4:
                            tensor.wait_ge(sAsc, it - 3)
                        for j in range(8):
                            ins = tensor.matmul(
                                psum[it % 4][:, j * D:(j + 1) * D],
                                xb[it % 4][:, j * 128:(j + 1) * 128],
                                WTs[:],
                                start=True, stop=True,
                            )
                        ins.then_inc(sAmm, 1)
                        continue
                    g = i
                    if g == 0:
                        tensor.wait_ge(sW, 32)
                    tensor.wait_ge(sBg[g % 4], (g // 4 + 1) * 16)
                    rhs = msg[g % 4][:].rearrange("p c d -> p (c d)")
                    for q in range(4):
                        if g >= 1:
                            tensor.wait_ge(sBcpS, 4 * (g - 1) + q + 1)
                        tensor.matmul(
                            psum[4 + q][:SPC, :],
                            SPs[:],
                            rhs[:, q * 512:(q + 1) * 512],
                            start=True, stop=True,
                        ).then_inc(sBmm, 1)

            @block.vector
            def _(vector):
                vector.wait_ge(sDeg, 32)
                vector.reciprocal(dinvTs[:], degTs[:]).then_inc(sDin, 1)
                vector.reciprocal(dinvDs[:], degDs[:]).then_inc(sDin, 1)
                vector.wait_ge(sDin, 3)  # sqrt done on scalar
                cnt = 0

                def accum(w):
                    nonlocal cnt
                    if w < NW - 1:
                        vector.wait_ge(sCa, (w + 1) * 16)
                        if w == 0:
                            vector.wait_ge(sCb, 16)
                            vector.tensor_tensor(
                                accs[:], Ab[0][:], Bb[0][:],
                                op=mybir.AluOpType.subtract,
                            ).then_inc(sCacc, 1)
                            cnt += 1
                        else:
                            vector.wait_ge(sCacc, cnt)
                            vector.tensor_tensor(
                                accs[:], accs[:], Ab[w % 2][:],
                                op=mybir.AluOpType.add,
                            ).then_inc(sCacc, 1)
                            cnt += 1
                            vector.wait_ge(sCb, (w + 1) * 16)
                            vector.wait_ge(sCacc, cnt)
                            vector.tensor_tensor(
                                accs[:], accs[:], Bb[w % 2][:],
                                op=mybir.AluOpType.subtract,
                            ).then_inc(sCacc, 1)
                            cnt += 1
                        vector.wait_ge(sCs, (w + 1) * 16)
                        vector.wait_ge(sCacc, cnt)
                        vector.tensor_tensor(
                            accs[:], accs[:], Sb[:], op=mybir.AluOpType.add
                        ).then_inc(sCacc, 1)
                        cnt += 1
                        return
                    # last window: half-split accumulate + finale
                    NH = NCOLS // 2
                    for h in range(2):
                        c0 = h * NH
                        c1 = NCOLS if h else NH
                        base = w * 16 + (h + 1) * 16
                        ah = accs[:, c0:c1, :]
                        vector.wait_ge(sCa, base)
                        vector.wait_ge(sCacc, cnt)
                        vector.tensor_tensor(
                            ah, ah, Ab[w % 2][:, c0:c1, :],
                            op=mybir.AluOpType.add,
                        ).then_inc(sCacc, 1)
                        cnt += 1
                        vector.wait_ge(sCb, base)
                        vector.wait_ge(sCacc, cnt)
                        vector.tensor_tensor(
                            ah, ah, Bb[w % 2][:, c0:c1, :],
                            op=mybir.AluOpType.subtract,
                        ).then_inc(sCacc, 1)
                        cnt += 1
                        vector.wait_ge(sCs, base)
                        vector.wait_ge(sCacc, cnt)
                        vector.tensor_tensor(
                            ah, ah, Sb[:, c0:c1, :],
                            op=mybir.AluOpType.add,
                        ).then_inc(sCacc, 1)
                        cnt += 1
                        dvDh = bcast(dinvDs[:, c0:c1], D)
                        vector.wait_ge(sCacc, cnt)
                        vector.tensor_tensor(
                            ah, ah, dvDh, op=mybir.AluOpType.mult
                        ).then_inc(sCacc, 1)
                        cnt += 1
                        apb = bBCs[:]
                        bbh = bass.AP(apb.tensor, apb.offset,
                                      [list(apb.ap[0]), [0, c1 - c0],
                                       list(apb.ap[1])])
                        vector.wait_ge(sLD3, 64)
                        vector.wait_ge(sCacc, cnt)
                        vector.tensor_tensor(
                            ah, ah, bbh, op=mybir.AluOpType.add
                        ).then_inc(sCacc, 1)
                        cnt += 1

                for it in range(AIT_REAL):
                    vector.wait_ge(sAmm, it + 1)
                    if it >= 4:
                        vector.wait_ge(sAout[it % 4],
                                       ((it - 4) // 4 + 1) * 16)
                    ps3 = psum[it % 4][:].rearrange("p (c d) -> p c d", d=D)
                    hb3 = hb[it % 4][:].rearrange("p (c d) -> p c d", d=D)
                    dv = bcast(dinvTs[:, it * 8:(it + 1) * 8], D)
                    vector.tensor_tensor(
                        hb3, ps3, dv, op=mybir.AluOpType.mult
                    ).then_inc(sAsc, 1)
                for w in range(NW):
                    accum(w)

            @block.scalar
            def _(scalar):
                scalar.wait_ge(sDin, 2)
                scalar.activation(dinvTs[:], dinvTs[:],
                                  mybir.ActivationFunctionType.Sqrt)
                scalar.activation(dinvDs[:], dinvDs[:],
                                  mybir.ActivationFunctionType.Sqrt
                                  ).then_inc(sDin, 1)
                # phase-C index tables + bias (needed from phase_c(0) on)
                scalar.dma_start(idxas[:], t_idxa[:]).then_inc(sLD3, 16)
                scalar.dma_start(idxbs[:], t_idxb[:]).then_inc(sLD3, 16)
                scalar.dma_start(idxss[:], t_idxs[:]).then_inc(sLD3, 16)
                scalar.dma_start(bBCs[:], t_bBC[:]).then_inc(sLD3, 16)
                # zero rows of the slot-prefix tables
                scalar.wait_ge(sFin, 1)
                for w in range(NW):
                    scalar.dma_start(
                        bass.AP(t_sp, SPOFF[w] * 128, [[128, 1], [1, D]]),
                        zrow[:]).then_inc(sCz, 16)
                gload_at = {Gsum[w]: w + 1 for w in range(1, NW - 1)}
                for g, (gw, gg) in enumerate(sched):
                    if g in gload_at:
                        # prefetch idx for window g+1 one window ahead;
                        # scalar position implies the target buffer's old
                        # readers (window wl-2 gathers) are long done.
                        wl = gload_at[g]
                        scalar.dma_start(
                            idx1s[:, (wl % 2) * IWC:
                                  (wl % 2) * IWC + G1s[wl] * IGC],
                            t_idx1[wl][:, :G1s[wl] * IGC],
                        ).then_inc(sBidx[wl % 2], 32)
                    if g >= 4:
                        scalar.wait_ge(sBst[g % 4], ((g - 4) // 4 + 1) * 16)
                    for q in range(4):
                        scalar.wait_ge(sBmm, 4 * g + q + 1)
                        scalar.activation(
                            sps[g % 4][:, q * 512:(q + 1) * 512],
                            psum[4 + q][:SPC, :],
                            mybir.ActivationFunctionType.Copy,
                        ).then_inc(sBcpS, 1)

            @block.gpsimd
            def _(gpsimd: bass.BassGpSimd):
                gpsimd.load_library(mlp)
                gpsimd.memset(zrow[:], 0.0).then_inc(sFin, 1)

                def staging(gs):
                    if gs < 0 or gs >= NG:
                        return
                    sgw, sgg = sched[gs]
                    gpsimd.wait_ge(sBcpS, 4 * gs + 4)
                    src3 = sps[gs % 4][:].rearrange("s (j d) -> s j d", d=D)
                    dst3 = bass.AP(
                        t_sp,
                        (SPOFF[sgw] + 1 + sgg * GCH * SPC) * 128,
                        [[GCH * 128, SPC], [128, GCH], [1, D]],
                    )
                    gpsimd.dma_start(dst3, src3).then_inc(sBst[gs % 4], 16)

                def phase_c(w):
                    # phase C for window w (after its staging lands)
                    for i in range(4):
                        gpsimd.wait_ge(sBst[i],
                                       ((Gsum[w + 1] + 3 - i) // 4) * 16)
                    if w == 0:
                        gpsimd.wait_ge(sLD3, 48)
                        gpsimd.wait_ge(sCz, NW * 16)
                    spw = bass.AP(t_sp, SPOFF[w] * 128,
                                  [[128, SPR[w]], [1, D]])
                    hp_w = bass.AP(t_hp, w * WSTRIDE * 128,
                                   [[128, WSTRIDE], [1, D]])
                    halves = 2 if w == NW - 1 else 1
                    NABH = NAB // halves
                    NOH = OUTROWS // halves
                    for h in range(halves):
                        c0 = h * (NCOLS // 2)
                        c1 = NCOLS if h else NCOLS // halves
                        if w >= 2 and h == 0:
                            gpsimd.wait_ge(sCacc, cnt_a[w - 2])
                        dma_gather_raw(
                            gpsimd, Ab[w % 2][:, c0:c1, :], spw,
                            idxas[:, w * NAB + h * NABH:
                                  w * NAB + (h + 1) * NABH],
                            NOH, NOH, D, 128, single_packet=False,
                        ).then_inc(sCa, 16)
                        if w >= 2 and h == 0:
                            gpsimd.wait_ge(sCacc, cnt_s[w - 2])
                        dma_gather_raw(
                            gpsimd, Bb[w % 2][:, c0:c1, :], spw,
                            idxbs[:, w * NAB + h * NABH:
                                  w * NAB + (h + 1) * NABH],
                            NOH, NOH, D, 128, single_packet=False,
                        ).then_inc(sCb, 16)
                        # self gather from h' window w
                        if w >= 1 and h == 0:
                            gpsimd.wait_ge(sCacc, cnt_s[w - 1])
                        dma_gather_raw(
                            gpsimd, Sb[:, c0:c1, :], hp_w,
                            idxss[:, w * NAB + h * NABH:
                                  w * NAB + (h + 1) * NABH],
                            NOH, NOH, D, 128, single_packet=False,
                        ).then_inc(sCs, 16)

                g = 0
                for w in range(NW):
                    # h' rows of window w ready
                    up = UPW[w]
                    for i in range(4):
                        gpsimd.wait_ge(sAout[i], ((up + 3 - i) // 4) * 16)
                    gpsimd.wait_ge(sBidx[w % 2], (w // 2 + 1) * 32)
                    ib = (w % 2) * IWC
                    hp_w = bass.AP(t_hp, w * WSTRIDE * 128,
                                   [[128, WSTRIDE], [1, D]])
                    for gg in range(G1s[w]):
                        if g >= 4:
                            gpsimd.wait_ge(sBmm, 4 * (g - 4) + 4)
                        dma_gather_raw(
                            gpsimd,
                            msg[g % 4][:], hp_w,
                            idx1s[:, ib + gg * IGC:ib + (gg + 1) * IGC],
                            NIDX, NIDX, D, 128, single_packet=False,
                        ).then_inc(sBg[g % 4], 16)
                        g += 1
                        staging(g - 3)
                        # phase C for the PREVIOUS window, a few groups in:
                        # its staging has landed and the current window's
                        # gather backlog keeps the DMA busy meanwhile.
                        if w >= 1 and gg == min(4, G1s[w] - 1):
                            phase_c(w - 1)
                for gs in range(max(NG - 3, 0), NG):
                    staging(gs)
                phase_c(NW - 1)

        nc.compile()
    return nc


def prepare(x, edge_index, W, b):
    x = np.asarray(x, dtype=np.float32)
    edge_index = np.asarray(edge_index)
    W = np.asarray(W, dtype=np.float32)
    b = np.asarray(b, dtype=np.float32)

    src = np.asarray(edge_index[0], dtype=np.int64)
    dst = np.asarray(edge_index[1], dtype=np.int64)

    deg = (np.bincount(dst, minlength=N) + 1.0).astype(np.float32)

    tbl_row = np.arange(N) + np.arange(N) // WCAP
    x_table = np.zeros((TBL, D), np.float16)
    x_table[tbl_row] = x.astype(np.float16)
    xT = np.ascontiguousarray(x_table.T)
    deg_table = np.ones(TBL, np.float32)
    deg_table[tbl_row] = deg
    degT = np.ascontiguousarray(deg_table.reshape(TBL // 128, 128).T)

    WT = np.ascontiguousarray(W.T.astype(np.float16))
    SP = (np.arange(128)[:, None] <= (L * np.arange(SPC) + L - 1)[None, :]
          ).astype(np.float16)
    bBC = np.ascontiguousarray(np.broadcast_to(b, (128, D))).astype(
        np.float32)

    shard_of = dst // SHARD
    order_c = np.argsort(shard_of, kind="stable")
    bounds = np.searchsorted(shard_of[order_c], np.arange(NCORES + 1))
    cores = []
    for c in range(NCORES):
        sel = order_c[bounds[c]:bounds[c + 1]]
        cores.append(_prep_core(src[sel], dst[sel] - c * SHARD))

    C1s = []
    for w in range(NW):
        cw = max(cr["C"][w] for cr in cores)
        C1s.append(max(GCH, -(-cw // GCH) * GCH))
    C1 = max(C1s)
    G1s = [c // GCH for c in C1s]
    SPR = [1 + c * SPC for c in C1s]
    assert max(SPR) <= 32767, f"slot-prefix table too large: {max(SPR)}"

    NAB = OUTROWS // 16
    in_maps = []
    for c in range(NCORES):
        cr = cores[c]
        idx1 = np.empty((NW, 64, C1 * 8), np.int16)
        idxa = np.empty((64, NW * NAB), np.int16)
        idxb = np.empty((64, NW * NAB), np.int16)
        idxs = np.empty((64, NW * NAB), np.int16)
        vown = np.arange(c * SHARD, (c + 1) * SHARD)
        rown = vown + vown // WCAP
        wown = rown // WSTRIDE
        for w in range(NW):
            fl = np.full(C1 * 128, ZROW, np.int64)
            fl[:cr["idx"][w].shape[0]] = cr["idx"][w]
            parts = [_wrap16(fl[g * NIDX:(g + 1) * NIDX])
                     for g in range(C1 // GCH)]
            idx1[w] = np.concatenate(parts, axis=1)
            for arr, dest in ((cr["a"][w], idxa), (cr["b"][w], idxb)):
                af = np.zeros(OUTROWS, np.int64)
                af[:SHARD] = arr
                dest[:, w * NAB:(w + 1) * NAB] = _wrap16(af)
            sf = np.full(OUTROWS, ZROW, np.int64)
            inw = wown == w
            sf[:SHARD][inw] = rown[inw] - w * WSTRIDE
            idxs[:, w * NAB:(w + 1) * NAB] = _wrap16(sf)
        degD_flat = np.ones(OUTROWS, np.float32)
        degD_flat[:SHARD] = deg[c * SHARD:(c + 1) * SHARD]
        degD = np.ascontiguousarray(degD_flat.reshape(NCOLS, 128).T)
        in_maps.append({
            "xT": xT, "WT": WT, "SP": SP, "degT": degT, "degD": degD,
            "bBC": bBC, "idx1": idx1, "idxa": idxa, "idxb": idxb,
            "idxs": idxs,
        })

    global SPR_G
    SPR_G = SPR
    nc = _build_program(C1, SPR, G1s)
    global LAST_NC
    LAST_NC = nc
    return nc, in_maps


def kernel(x, edge_index, W, b):
    nc, in_maps = prepare(x, edge_index, W, b)
    res = bass_utils.run_bass_kernel_spmd(nc, in_maps,
                                          core_ids=list(range(NCORES)))
    out = np.empty((N, D), np.float32)
    for c in range(NCORES):
        out[c * SHARD:(c + 1) * SHARD] = \
            res.results[c]["out_s"][:SHARD].astype(np.float32)
    return out


# revision 36
# speedup vs baseline: 2.0474x; 1.0047x over previous
"""GCNConv (PyG semantics) on 8 Trainium2 NeuronCores.

out = D^-1/2 (A+I) D^-1/2 (x @ W.T) + b, dst-sharded across 8 cores.

Host: bucket edges by dst shard; split each node's edge list by 4 equal
source windows of 25,087 rows (dma_gather indices are int16); pad each
(node, window) run into slots of L=4 edges; best-fit-decreasing pack slot
runs into 128-edge chunks. Self-loops are NOT edges: they are gathered
directly from the h' table by 4 per-window "self" gathers.

Device, per core (all message data fp16; 128-byte gather descriptors):
  Phase A: h' = (x @ W.T) * dinv[row] over a 100352-row padded table
           (1 zero row per window); written to HBM as fp16 rows strided
           256B ([TBL, 128], payload cols 0:64).
  Phase B: per window, dma_gather 4096 messages at a time from h'; PE
           matmul with the slot-prefix matrix SP [128, 32] -> PSUM
           (slot-prefix sums); DVE/ACT copy to SBUF fp16; DMA-stage to a
           per-window DRAM slot-prefix table (row 0 reserved zero).
  Phase C: per window, one dma_gather per boundary side (a = last slot,
           b = before first slot) into double-buffered fp16 tiles, plus a
           self gather from h'; acc += A_w - B_w + S_w (fp16); then scale
           by dinv[dst], add bias, write out shard fp16 (host -> fp32).
"""

import numpy as np
from contextlib import ExitStack

import concourse.bacc as bacc
import concourse.bass as bass
import concourse.mybir as mybir
from concourse import ap_utils, bass_utils
from concourse.bass import MemorySpace
from concourse.library_config import mlp

D = 64


def configure(n=100000, ncores=8, wcap=25087, nw=4, slot=4, gch=32):
    # set problem geometry (module globals); defaults = real problem
    global N, NCORES, SHARD, WCAP, NW, WSTRIDE, TBL, ZROW
    global OUTROWS, NCOLS, AIT, AIT_REAL
    global L, SPC, GCH, NIDX, CLIM
    N = n
    NCORES = ncores
    SHARD = N // NCORES
    WCAP = wcap
    NW = nw
    WSTRIDE = WCAP + 1
    assert WSTRIDE % 128 == 0 and NW * WCAP >= N
    TBL = NW * WSTRIDE
    ZROW = WCAP
    OUTROWS = -(-SHARD // 128) * 128
    NCOLS = OUTROWS // 128
    assert TBL % 1024 == 0
    AIT = TBL // 1024
    max_row = (N - 1) + (N - 1) // WCAP
    AIT_REAL = -(-(max_row + 1) // 1024)
    L = slot
    SPC = 128 // L         # slot-prefix columns per chunk
    GCH = gch
    NIDX = GCH * 128
    # per-window staging table must stay int16-addressable
    CLIM = (32767 - 1) // SPC


configure()
LAST_NC = None


def dma_gather_raw(eng, out_ap, in_ap, idxs_ap, num_idxs, num_idxs_reg,
                   elem_size, elem_step, single_packet=False, queue_num=0):
    """bass.BassGpSimd.dma_gather minus the elem%256B assert (non-transpose
    HBM source; the ucode packetizes any elem size — only the table STRIDE
    must be a multiple of 256B). Verified on HW with 128B fp16 elems."""
    assert idxs_ap.dtype == mybir.dt.int16
    assert in_ap.dtype == out_ap.dtype
    assert in_ap.space == MemorySpace.DRAM
    assert idxs_ap.space == MemorySpace.SBUF
    assert out_ap.space == MemorySpace.SBUF
    assert ap_utils.ap_is_contiguous(in_ap.ap[1:])
    assert ap_utils.ap_is_contiguous(out_ap.ap[1:])
    assert ap_utils.ap_is_contiguous(idxs_ap.ap[1:])
    assert in_ap.ap[-1][1] == out_ap.ap[-1][1] == elem_size
    assert out_ap.ap[0][1] * out_ap.ap[1][1] == bass.round_up_to_multiple(
        num_idxs, 128)
    assert in_ap.ap[0][0] == elem_step
    stride_bytes = elem_step * mybir.dt.size(in_ap.dtype)
    stride_bytes_256 = bass.exact_div(stride_bytes, 256)
    assert stride_bytes_256 < 256
    return eng.add_instruction(
        mybir.InstDMAGatherAnt(
            name=eng.bass.get_next_instruction_name(),
            ins=[*eng.lower_ap_dma(in_ap, for_custom_bir_dma=True),
                 eng.lower_ap(idxs_ap),
                 eng.lower_val_access(eng.to_reg(num_idxs_reg))],
            outs=[eng.lower_ap(out_ap)],
            transpose=False,
            num_idxs=num_idxs,
            elem_size=elem_size,
            stride_bytes_256=stride_bytes_256,
            gen_mode=0,
            single_packet=single_packet,
            queue_num=queue_num,
            sbuf_tokens_per_rank=0,
            sbuf_free_dim_per_rank=0,
            sbuf_free_dim_pad_per_rank=0,
            sbuf_byte_offset=0,
        )
    )


def _wrap16(idx_flat):
    """Flat idx list -> dma_gather int16 wrap [16, n//16] tiled to 64."""
    n = idx_flat.shape[0]
    out = idx_flat.reshape(n // 16, 16).T.astype(np.int16)
    return np.tile(out, (4, 1))


def _pack_cell(counts, Lc):
    """Best-fit-ish decreasing pack of ceil(counts/Lc) slot runs into
    128-edge chunks (capacity 128//Lc slots)."""
    cap = 128 // Lc
    nz = np.nonzero(counts)[0]
    sizes = -(-counts[nz] // Lc)
    assert (sizes <= cap).all()
    smax = int(sizes.max()) if nz.size else 1
    by_size = [np.nonzero(sizes == s)[0] for s in range(smax + 1)]
    ptr = [0] * (smax + 1)
    remaining = int(nz.size)
    chunk_of = np.zeros(nz.size, np.int64)
    slot_start = np.zeros(nz.size, np.int64)
    cur = 0
    while remaining:
        space = cap
        while True:
            s = min(space, smax)
            while s > 0 and ptr[s] >= len(by_size[s]):
                s -= 1
            if s == 0:
                break
            k = by_size[s][ptr[s]]
            ptr[s] += 1
            chunk_of[k] = cur
            slot_start[k] = cap - space
            space -= s
            remaining -= 1
        cur += 1
    return nz, sizes, chunk_of, slot_start, cur


def _prep_core(src_g, dst_l):
    """Per-core, per-window gather/aggregation structures."""
    win = src_g // WCAP
    loc = src_g - win * WCAP  # 0..WCAP-1

    res = {"idx": [], "a": [], "b": [], "C": []}
    for w in range(NW):
        m = win == w
        dw = dst_l[m]
        lw = loc[m]
        order = np.argsort(dw, kind="stable")
        dw = dw[order]
        lw = lw[order]
        counts = np.bincount(dw, minlength=SHARD)
        nz, sizes, chunk_of, slot_start, C = _pack_cell(counts, L)
        assert C <= CLIM, f"cell needs {C} chunks > {CLIM}"
        starts = np.zeros(SHARD, np.int64)
        starts[1:] = np.cumsum(counts)[:-1]
        pos_base = chunk_of * 128 + slot_start * L
        idx = np.full(max(C, 1) * 128, ZROW, np.int64)
        within = np.arange(dw.shape[0]) - np.repeat(starts[nz], counts[nz])
        pos = np.repeat(pos_base, counts[nz]) + within
        idx[pos] = lw

        def sprow(ch, col):
            return (ch // GCH) * (GCH * SPC) + col * GCH + ch % GCH + 1

        a = np.zeros(SHARD, np.int64)
        b = np.zeros(SHARD, np.int64)
        a[nz] = sprow(chunk_of, slot_start + sizes - 1)
        sb0 = slot_start > 0
        bnz = np.zeros(nz.size, np.int64)
        bnz[sb0] = sprow(chunk_of[sb0], slot_start[sb0] - 1)
        b[nz] = bnz
        res["idx"].append(idx)
        res["a"].append(a)
        res["b"].append(b)
        res["C"].append(C)
    return res


def _build_program(C1, SPR, G1s, CTRUE):
    sched = [(w, gg) for w in range(NW) for gg in range(G1s[w])]

    def chg(w, gg):
        if gg == G1s[w] - 1:
            return CTRUE[w] - (G1s[w] - 1) * GCH
        return GCH
    NG = len(sched)             # total phase-B groups
    Gsum = [0]
    for w in range(NW):
        Gsum.append(Gsum[-1] + G1s[w])
    SPOFF = [0]                 # per-window staging table row offsets
    for w in range(NW):
        SPOFF.append(SPOFF[-1] + SPR[w])
    dt = mybir.dt
    TCOL = TBL // 128
    NAB = OUTROWS // 16         # idx cols per (window, a|b|s)
    IGC = NIDX // 16            # idx cols per phase-B group
    IWC = C1 * 8                # idx cols per window
    # h'-write iterations covering window w
    UPW = [min(-(-((w + 1) * WSTRIDE) // 1024), AIT_REAL) for w in range(NW)]
    # static interleave of phase-A iterations and phase-B groups on PE/DVE:
    # B-group g becomes eligible ~2 A-iters after its gather could start.
    ELIG = [min(UPW[gw] + 2 * (g - Gsum[gw]) + 2, AIT_REAL)
            for g, (gw, gg) in enumerate(sched)]
    MERGED = []
    _gq = 0
    for _it in range(AIT_REAL):
        MERGED.append(("A", _it))
        while _gq < len(sched) and ELIG[_gq] <= _it + 1:
            MERGED.append(("B", _gq))
            _gq += 1
    while _gq < len(sched):
        MERGED.append(("B", _gq))
        _gq += 1
    # consumed-op counters: per window ops (+A/-B or A-B), +S
    cnt_a = [1, 3, 6, 9][:NW]       # op index when A_w consumed
    cnt_b = [1, 4, 7, 10][:NW]      # op index when B_w consumed
    cnt_s = [2, 5, 8, 11][:NW]      # op index when S_w consumed

    nc = bacc.Bacc("TRN2", target_bir_lowering=False, debug=False,
                   num_devices=NCORES)
    t_xT = nc.dram_tensor("xT", [D, TBL], dt.float16, kind="ExternalInput")
    t_WT = nc.dram_tensor("WT", [D, D], dt.float16, kind="ExternalInput")
    t_SP = nc.dram_tensor("SP", [128, SPC], dt.float16,
                          kind="ExternalInput")
    t_degT = nc.dram_tensor("degT", [128, TCOL], dt.float32,
                            kind="ExternalInput")
    t_degD = nc.dram_tensor("degD", [128, NCOLS], dt.float32,
                            kind="ExternalInput")
    t_bBC = nc.dram_tensor("bBC", [128, D], dt.float32, kind="ExternalInput")
    t_idx1 = nc.dram_tensor("idx1", [NW, 64, IWC], dt.int16,
                            kind="ExternalInput")
    t_idxa = nc.dram_tensor("idxa", [64, NW * NAB], dt.int16,
                            kind="ExternalInput")
    t_idxb = nc.dram_tensor("idxb", [64, NW * NAB], dt.int16,
                            kind="ExternalInput")
    t_idxs = nc.dram_tensor("idxs", [64, NW * NAB], dt.int16,
                            kind="ExternalInput")
    t_hp = nc.dram_tensor("hp", [TBL, 128], dt.float16)
    t_sp = nc.dram_tensor("sp", [SPOFF[NW], 128], dt.float16)
    t_out = nc.dram_tensor("out_s", [OUTROWS, D], dt.float16,
                           kind="ExternalOutput")

    with ExitStack() as ctx:
        e = ctx.enter_context
        xb = [e(nc.sbuf_tensor(f"xb{i}", [D, 1024], dt.float16))
              for i in range(4)]
        hb = [e(nc.sbuf_tensor(f"hb{i}", [128, 512], dt.float16))
              for i in range(4)]
        WTs = e(nc.sbuf_tensor("WTs", [D, D], dt.float16))
        SPs = e(nc.sbuf_tensor("SPs", [128, SPC], dt.float16))
        bBCs = e(nc.sbuf_tensor("bBCs", [128, D], dt.float32))
        degTs = e(nc.sbuf_tensor("degTs", [128, TCOL], dt.float32))
        dinvTs = e(nc.sbuf_tensor("dinvTs", [128, TCOL], dt.float32))
        degDs = e(nc.sbuf_tensor("degDs", [128, NCOLS], dt.float32))
        dinvDs = e(nc.sbuf_tensor("dinvDs", [128, NCOLS], dt.float32))
        idx1s = e(nc.sbuf_tensor("idx1s", [64, 2 * IWC], dt.int16))
        idxas = e(nc.sbuf_tensor("idxas", [64, NW * NAB], dt.int16))
        idxbs = e(nc.sbuf_tensor("idxbs", [64, NW * NAB], dt.int16))
        idxss = e(nc.sbuf_tensor("idxss", [64, NW * NAB], dt.int16))
        msg = [e(nc.sbuf_tensor(f"msg{i}", [128, GCH, D], dt.float16))
               for i in range(4)]
        sps = [e(nc.sbuf_tensor(f"sps{i}", [SPC, GCH * D], dt.float16))
               for i in range(4)]
        zrow = e(nc.sbuf_tensor("zrow", [1, D], dt.float16))
        Ab = [e(nc.sbuf_tensor(f"Ab{i}", [128, NCOLS, D], dt.float16))
              for i in range(2)]
        Bb = [e(nc.sbuf_tensor(f"Bb{i}", [128, NCOLS, D], dt.float16))
              for i in range(2)]
        Sb = e(nc.sbuf_tensor("Sb", [128, NCOLS, D], dt.float16))
        accs = e(nc.sbuf_tensor("accs", [128, NCOLS, D], dt.float16))
        psum = [e(nc.psum_tensor(f"ps{i}", [128, 512], dt.float32))
                for i in range(8)]

        sW = e(nc.semaphore("sW"))
        sDeg = e(nc.semaphore("sDeg"))
        sLD3 = e(nc.semaphore("sLD3"))
        sAx = [e(nc.semaphore(f"sAx{i}")) for i in range(4)]
        sAmm = e(nc.semaphore("sAmm"))
        sAsc = e(nc.semaphore("sAsc"))
        sAout = [e(nc.semaphore(f"sAout{i}")) for i in range(4)]
        sDin = e(nc.semaphore("sDin"))
        sBidx = [e(nc.semaphore(f"sBidx{i}")) for i in range(2)]
        sBg = [e(nc.semaphore(f"sBg{i}")) for i in range(4)]
        sBmm = e(nc.semaphore("sBmm"))
        sBcpV = e(nc.semaphore("sBcpV"))
        sBcpS = e(nc.semaphore("sBcpS"))
        sBst = [e(nc.semaphore(f"sBst{i}")) for i in range(4)]
        sCz = e(nc.semaphore("sCz"))
        sCa = e(nc.semaphore("sCa"))
        sCb = e(nc.semaphore("sCb"))
        sCs = e(nc.semaphore("sCs"))
        sCacc = e(nc.semaphore("sCacc"))
        sFin = e(nc.semaphore("sFin"))

        def bcast(ap, reps):
            return bass.AP(ap.tensor, ap.offset, list(ap.ap) + [[0, reps]])

        with nc.Block() as block:

            @block.sync
            def _(sync: bass.BassEngine):
                # x tile 0 first so phase A starts immediately
                sync.dma_start(xb[0][:], t_xT[:, 0:1024]).then_inc(sAx[0], 16)
                sync.dma_start(WTs[:], t_WT[:]).then_inc(sW, 16)
                sync.dma_start(SPs[:], t_SP[:]).then_inc(sW, 16)
                sync.dma_start(degTs[:], t_degT[:]).then_inc(sDeg, 16)
                sync.dma_start(degDs[:], t_degD[:]).then_inc(sDeg, 16)
                sync.dma_start(idx1s[:, 0:G1s[0] * IGC],
                               t_idx1[0][:, :G1s[0] * IGC]
                               ).then_inc(sBidx[0], 32)
                sync.dma_start(idx1s[:, IWC:IWC + G1s[1] * IGC],
                               t_idx1[1][:, :G1s[1] * IGC]
                               ).then_inc(sBidx[1], 32)
                # phase A, interleaved x-in / h'-out
                for it in range(AIT_REAL + 2):
                    if 0 < it < AIT_REAL:
                        if it >= 4:
                            sync.wait_ge(sAmm, it - 3)
                        sync.dma_start(
                            xb[it % 4][:], t_xT[:, it * 1024:(it + 1) * 1024]
                        ).then_inc(sAx[it % 4], 16)
                    if it >= 2:
                        jo = it - 2
                        sync.wait_ge(sAsc, jo + 1)
                        src3 = hb[jo % 4][:].rearrange("p (c d) -> p c d",
                                                       d=D)
                        dst3 = bass.AP(t_hp, jo * 1024 * 128,
                                       [[128, 128], [128 * 128, 8], [1, D]])
                        sync.dma_start(dst3, src3).then_inc(sAout[jo % 4], 16)
                # final out (two column halves to shorten the tail)
                NH = NCOLS // 2
                sync.wait_ge(sCacc, 3 * NW - 4 + 5)
                o3a = bass.AP(t_out, 0, [[D, 128], [128 * D, NH], [1, D]])
                sync.dma_start(o3a, accs[:, 0:NH, :]).then_inc(sFin, 16)
                sync.wait_ge(sCacc, 3 * NW - 4 + 10)
                o3b = bass.AP(t_out, NH * 128 * D,
                              [[D, 128], [128 * D, NCOLS - NH], [1, D]])
                sync.dma_start(o3b, accs[:, NH:NCOLS, :]).then_inc(sFin, 16)
                sync.wait_ge(sFin, 33)

            @block.tensor
            def _(tensor):
                tensor.wait_ge(sW, 16)
                for kind, i in MERGED:
                    if kind == "A":
                        it = i
                        tensor.wait_ge(sAx[it % 4], (it // 4 + 1) * 16)
                        if it >= 4:
                            tensor.wait_ge(sAsc, it - 3)
                        for j in range(8):
                            ins = tensor.matmul(
                                psum[it % 4][:, j * D:(j + 1) * D],
                                xb[it % 4][:, j * 128:(j + 1) * 128],
                                WTs[:],
                                start=True, stop=True,
                            )
                        ins.then_inc(sAmm, 1)
                        continue
                    g = i
                    if g == 0:
                        tensor.wait_ge(sW, 32)
                    tensor.wait_ge(sBg[g % 4], (g // 4 + 1) * 16)
                    rhs = msg[g % 4][:].rearrange("p c d -> p (c d)")
                    for q in range(4):
                        if g >= 1:
                            tensor.wait_ge(sBcpS, 4 * (g - 1) + q + 1)
                        tensor.matmul(
                            psum[4 + q][:SPC, :],
                            SPs[:],
                            rhs[:, q * 512:(q + 1) * 512],
                            start=True, stop=True,
                        ).then_inc(sBmm, 1)

            @block.vector
            def _(vector):
                vector.wait_ge(sDeg, 32)
                vector.reciprocal(dinvTs[:], degTs[:]).then_inc(sDin, 1)
                vector.reciprocal(dinvDs[:], degDs[:]).then_inc(sDin, 1)
                vector.wait_ge(sDin, 3)  # sqrt done on scalar
                cnt = 0

                def accum(w):
                    nonlocal cnt
                    if w < NW - 1:
                        vector.wait_ge(sCa, (w + 1) * 16)
                        if w == 0:
                            vector.wait_ge(sCb, 16)
                            vector.tensor_tensor(
                                accs[:], Ab[0][:], Bb[0][:],
                                op=mybir.AluOpType.subtract,
                            ).then_inc(sCacc, 1)
                            cnt += 1
                        else:
                            vector.wait_ge(sCacc, cnt)
                            vector.tensor_tensor(
                                accs[:], accs[:], Ab[w % 2][:],
                                op=mybir.AluOpType.add,
                            ).then_inc(sCacc, 1)
                            cnt += 1
                            vector.wait_ge(sCb, (w + 1) * 16)
                            vector.wait_ge(sCacc, cnt)
                            vector.tensor_tensor(
                                accs[:], accs[:], Bb[w % 2][:],
                                op=mybir.AluOpType.subtract,
                            ).then_inc(sCacc, 1)
                            cnt += 1
                        vector.wait_ge(sCs, (w + 1) * 16)
                        vector.wait_ge(sCacc, cnt)
                        vector.tensor_tensor(
                            accs[:], accs[:], Sb[:], op=mybir.AluOpType.add
                        ).then_inc(sCacc, 1)
                        cnt += 1
                        return
                    # last window: half-split accumulate + finale
                    NH = NCOLS // 2
                    for h in range(2):
                        c0 = h * NH
                        c1 = NCOLS if h else NH
                        base = w * 16 + (h + 1) * 16
                        ah = accs[:, c0:c1, :]
                        vector.wait_ge(sCa, base)
                        vector.wait_ge(sCacc, cnt)
                        vector.tensor_tensor(
                            ah, ah, Ab[w % 2][:, c0:c1, :],
                            op=mybir.AluOpType.add,
                        ).then_inc(sCacc, 1)
                        cnt += 1
                        vector.wait_ge(sCb, base)
                        vector.wait_ge(sCacc, cnt)
                        vector.tensor_tensor(
                            ah, ah, Bb[w % 2][:, c0:c1, :],
                            op=mybir.AluOpType.subtract,
                        ).then_inc(sCacc, 1)
                        cnt += 1
                        vector.wait_ge(sCs, base)
                        vector.wait_ge(sCacc, cnt)
                        vector.tensor_tensor(
                            ah, ah, Sb[:, c0:c1, :],
                            op=mybir.AluOpType.add,
                        ).then_inc(sCacc, 1)
                        cnt += 1
                        dvDh = bcast(dinvDs[:, c0:c1], D)
                        vector.wait_ge(sCacc, cnt)
                        vector.tensor_tensor(
                            ah, ah, dvDh, op=mybir.AluOpType.mult
                        ).then_inc(sCacc, 1)
                        cnt += 1
                        apb = bBCs[:]
                        bbh = bass.AP(apb.tensor, apb.offset,
                                      [list(apb.ap[0]), [0, c1 - c0],
                                       list(apb.ap[1])])
                        vector.wait_ge(sLD3, 64)
                        vector.wait_ge(sCacc, cnt)
                        vector.tensor_tensor(
                            ah, ah, bbh, op=mybir.AluOpType.add
                        ).then_inc(sCacc, 1)
                        cnt += 1

                for it in range(AIT_REAL):
                    vector.wait_ge(sAmm, it + 1)
                    if it >= 4:
                        vector.wait_ge(sAout[it % 4],
                                       ((it - 4) // 4 + 1) * 16)
                    ps3 = psum[it % 4][:].rearrange("p (c d) -> p c d", d=D)
                    hb3 = hb[it % 4][:].rearrange("p (c d) -> p c d", d=D)
                    dv = bcast(dinvTs[:, it * 8:(it + 1) * 8], D)
                    vector.tensor_tensor(
                        hb3, ps3, dv, op=mybir.AluOpType.mult
                    ).then_inc(sAsc, 1)
                for w in range(NW):
                    accum(w)

            @block.scalar
            def _(scalar):
                scalar.wait_ge(sDin, 2)
                scalar.activation(dinvTs[:], dinvTs[:],
                                  mybir.ActivationFunctionType.Sqrt)
                scalar.activation(dinvDs[:], dinvDs[:],
                                  mybir.ActivationFunctionType.Sqrt
                                  ).then_inc(sDin, 1)
                # phase-C index tables + bias (needed from phase_c(0) on)
                scalar.dma_start(idxas[:], t_idxa[:]).then_inc(sLD3, 16)
                scalar.dma_start(idxbs[:], t_idxb[:]).then_inc(sLD3, 16)
                scalar.dma_start(idxss[:], t_idxs[:]).then_inc(sLD3, 16)
                scalar.dma_start(bBCs[:], t_bBC[:]).then_inc(sLD3, 16)
                # zero rows of the slot-prefix tables
                scalar.wait_ge(sFin, 1)
                for w in range(NW):
                    scalar.dma_start(
                        bass.AP(t_sp, SPOFF[w] * 128, [[128, 1], [1, D]]),
                        zrow[:]).then_inc(sCz, 16)
                gload_at = {Gsum[w]: w + 1 for w in range(1, NW - 1)}
                for g, (gw, gg) in enumerate(sched):
                    if g in gload_at:
                        # prefetch idx for window g+1 one window ahead;
                        # scalar position implies the target buffer's old
                        # readers (window wl-2 gathers) are long done.
                        wl = gload_at[g]
                        scalar.dma_start(
                            idx1s[:, (wl % 2) * IWC:
                                  (wl % 2) * IWC + G1s[wl] * IGC],
                            t_idx1[wl][:, :G1s[wl] * IGC],
                        ).then_inc(sBidx[wl % 2], 32)
                    if g >= 4:
                        scalar.wait_ge(sBst[g % 4], ((g - 4) // 4 + 1) * 16)
                    for q in range(4):
                        scalar.wait_ge(sBmm, 4 * g + q + 1)
                        scalar.activation(
                            sps[g % 4][:, q * 512:(q + 1) * 512],
                            psum[4 + q][:SPC, :],
                            mybir.ActivationFunctionType.Copy,
                        ).then_inc(sBcpS, 1)

            @block.gpsimd
            def _(gpsimd: bass.BassGpSimd):
                gpsimd.load_library(mlp)
                gpsimd.memset(zrow[:], 0.0).then_inc(sFin, 1)

                def staging(gs):
                    if gs < 0 or gs >= NG:
                        return
                    sgw, sgg = sched[gs]
                    nch = chg(sgw, sgg)
                    gpsimd.wait_ge(sBcpS, 4 * gs + 4)
                    src3 = sps[gs % 4][:].rearrange("s (j d) -> s j d",
                                                    d=D)[:, 0:nch, :]
                    dst3 = bass.AP(
                        t_sp,
                        (SPOFF[sgw] + 1 + sgg * GCH * SPC) * 128,
                        [[GCH * 128, SPC], [128, nch], [1, D]],
                    )
                    gpsimd.dma_start(dst3, src3).then_inc(sBst[gs % 4], 16)

                def phase_c(w):
                    # phase C for window w (after its staging lands)
                    for i in range(4):
                        gpsimd.wait_ge(sBst[i],
                                       ((Gsum[w + 1] + 3 - i) // 4) * 16)
                    if w == 0:
                        gpsimd.wait_ge(sLD3, 48)
                        gpsimd.wait_ge(sCz, NW * 16)
                    spw = bass.AP(t_sp, SPOFF[w] * 128,
                                  [[128, SPR[w]], [1, D]])
                    hp_w = bass.AP(t_hp, w * WSTRIDE * 128,
                                   [[128, WSTRIDE], [1, D]])
                    halves = 2 if w == NW - 1 else 1
                    NABH = NAB // halves
                    NOH = OUTROWS // halves
                    for h in range(halves):
                        c0 = h * (NCOLS // 2)
                        c1 = NCOLS if h else NCOLS // halves
                        if w >= 2 and h == 0:
                            gpsimd.wait_ge(sCacc, cnt_a[w - 2])
                        dma_gather_raw(
                            gpsimd, Ab[w % 2][:, c0:c1, :], spw,
                            idxas[:, w * NAB + h * NABH:
                                  w * NAB + (h + 1) * NABH],
                            NOH, NOH, D, 128, single_packet=False,
                        ).then_inc(sCa, 16)
                        if w >= 2 and h == 0:
                            gpsimd.wait_ge(sCacc, cnt_s[w - 2])
                        dma_gather_raw(
                            gpsimd, Bb[w % 2][:, c0:c1, :], spw,
                            idxbs[:, w * NAB + h * NABH:
                                  w * NAB + (h + 1) * NABH],
                            NOH, NOH, D, 128, single_packet=False,
                        ).then_inc(sCb, 16)
                        # self gather from h' window w
                        if w >= 1 and h == 0:
                            gpsimd.wait_ge(sCacc, cnt_s[w - 1])
                        dma_gather_raw(
                            gpsimd, Sb[:, c0:c1, :], hp_w,
                            idxss[:, w * NAB + h * NABH:
                                  w * NAB + (h + 1) * NABH],
                            NOH, NOH, D, 128, single_packet=False,
                        ).then_inc(sCs, 16)

                g = 0
                for w in range(NW):
                    # h' rows of window w ready
                    up = UPW[w]
                    for i in range(4):
                        gpsimd.wait_ge(sAout[i], ((up + 3 - i) // 4) * 16)
                    gpsimd.wait_ge(sBidx[w % 2], (w // 2 + 1) * 32)
                    ib = (w % 2) * IWC
                    hp_w = bass.AP(t_hp, w * WSTRIDE * 128,
                                   [[128, WSTRIDE], [1, D]])
                    for gg in range(G1s[w]):
                        if g >= 4:
                            gpsimd.wait_ge(sBmm, 4 * (g - 4) + 4)
                        nch = chg(w, gg)
                        dma_gather_raw(
                            gpsimd,
                            msg[g % 4][:, 0:nch, :], hp_w,
                            idx1s[:, ib + gg * IGC:
                                  ib + gg * IGC + nch * 8],
                            nch * 128, nch * 128, D, 128,
                            single_packet=False,
                        ).then_inc(sBg[g % 4], 16)
                        g += 1
                        staging(g - 3)
                        # phase C for the PREVIOUS window, a few groups in:
                        # its staging has landed and the current window's
                        # gather backlog keeps the DMA busy meanwhile.
                        if w >= 1 and gg == min(4, G1s[w] - 1):
                            phase_c(w - 1)
                for gs in range(max(NG - 3, 0), NG):
                    staging(gs)
                phase_c(NW - 1)

        nc.compile()
    return nc


def prepare(x, edge_index, W, b):
    x = np.asarray(x, dtype=np.float32)
    edge_index = np.asarray(edge_index)
    W = np.asarray(W, dtype=np.float32)
    b = np.asarray(b, dtype=np.float32)

    src = np.asarray(edge_index[0], dtype=np.int64)
    dst = np.asarray(edge_index[1], dtype=np.int64)

    deg = (np.bincount(dst, minlength=N) + 1.0).astype(np.float32)

    tbl_row = np.arange(N) + np.arange(N) // WCAP
    x_table = np.zeros((TBL, D), np.float16)
    x_table[tbl_row] = x.astype(np.float16)
    xT = np.ascontiguousarray(x_table.T)
    deg_table = np.ones(TBL, np.float32)
    deg_table[tbl_row] = deg
    degT = np.ascontiguousarray(deg_table.reshape(TBL // 128, 128).T)

    WT = np.ascontiguousarray(W.T.astype(np.float16))
    SP = (np.arange(128)[:, None] <= (L * np.arange(SPC) + L - 1)[None, :]
          ).astype(np.float16)
    bBC = np.ascontiguousarray(np.broadcast_to(b, (128, D))).astype(
        np.float32)

    shard_of = dst // SHARD
    order_c = np.argsort(shard_of, kind="stable")
    bounds = np.searchsorted(shard_of[order_c], np.arange(NCORES + 1))
    cores = []
    for c in range(NCORES):
        sel = order_c[bounds[c]:bounds[c + 1]]
        cores.append(_prep_core(src[sel], dst[sel] - c * SHARD))

    C1s = []
    CTRUE = []
    for w in range(NW):
        cw = max(cr["C"][w] for cr in cores)
        CTRUE.append(max(cw, 1))
        C1s.append(max(GCH, -(-cw // GCH) * GCH))
    C1 = max(C1s)
    G1s = [c // GCH for c in C1s]
    SPR = [1 + c * SPC for c in C1s]
    assert max(SPR) <= 32767, f"slot-prefix table too large: {max(SPR)}"

    NAB = OUTROWS // 16
    in_maps = []
    for c in range(NCORES):
        cr = cores[c]
        idx1 = np.empty((NW, 64, C1 * 8), np.int16)
        idxa = np.empty((64, NW * NAB), np.int16)
        idxb = np.empty((64, NW * NAB), np.int16)
        idxs = np.empty((64, NW * NAB), np.int16)
        vown = np.arange(c * SHARD, (c + 1) * SHARD)
        rown = vown + vown // WCAP
        wown = rown // WSTRIDE
        for w in range(NW):
            fl = np.full(C1 * 128, ZROW, np.int64)
            fl[:cr["idx"][w].shape[0]] = cr["idx"][w]
            parts = [_wrap16(fl[g * NIDX:(g + 1) * NIDX])
                     for g in range(C1 // GCH)]
            idx1[w] = np.concatenate(parts, axis=1)
            for arr, dest in ((cr["a"][w], idxa), (cr["b"][w], idxb)):
                af = np.zeros(OUTROWS, np.int64)
                af[:SHARD] = arr
                dest[:, w * NAB:(w + 1) * NAB] = _wrap16(af)
            sf = np.full(OUTROWS, ZROW, np.int64)
            inw = wown == w
            sf[:SHARD][inw] = rown[inw] - w * WSTRIDE
            idxs[:, w * NAB:(w + 1) * NAB] = _wrap16(sf)
        degD_flat = np.ones(OUTROWS, np.float32)
        degD_flat[:SHARD] = deg[c * SHARD:(c + 1) * SHARD]
        degD = np.ascontiguousarray(degD_flat.reshape(NCOLS, 128).T)
        in_maps.append({
            "xT": xT, "WT": WT, "SP": SP, "degT": degT, "degD": degD,
            "bBC": bBC, "idx1": idx1, "idxa": idxa, "idxb": idxb,
            "idxs": idxs,
        })

    global SPR_G
    SPR_G = SPR
    nc = _build_program(C1, SPR, G1s, CTRUE)
    global LAST_NC
    LAST_NC = nc
    return nc, in_maps


def kernel(x, edge_index, W, b):
    nc, in_maps = prepare(x, edge_index, W, b)
    res = bass_utils.run_bass_kernel_spmd(nc, in_maps,
                                          core_ids=list(range(NCORES)))
    out = np.empty((N, D), np.float32)
    for c in range(NCORES):
        out[c * SHARD:(c + 1) * SHARD] = \
            res.results[c]["out_s"][:SHARD].astype(np.float32)
    return out


# revision 38
# speedup vs baseline: 2.0658x; 1.0090x over previous
"""GCNConv (PyG semantics) on 8 Trainium2 NeuronCores.

out = D^-1/2 (A+I) D^-1/2 (x @ W.T) + b, dst-sharded across 8 cores.

Host: bucket edges by dst shard; split each node's edge list by 4 equal
source windows of 25,087 rows (dma_gather indices are int16); pad each
(node, window) run into slots of L=4 edges; best-fit-decreasing pack slot
runs into 128-edge chunks. Self-loops are NOT edges: they are gathered
directly from the h' table by 4 per-window "self" gathers.

Device, per core (all message data fp16; 128-byte gather descriptors):
  Phase A: h' = (x @ W.T) * dinv[row] over a 100352-row padded table
           (1 zero row per window); written to HBM as fp16 rows strided
           256B ([TBL, 128], payload cols 0:64).
  Phase B: per window, dma_gather 4096 messages at a time from h'; PE
           matmul with the slot-prefix matrix SP [128, 32] -> PSUM
           (slot-prefix sums); DVE/ACT copy to SBUF fp16; DMA-stage to a
           per-window DRAM slot-prefix table (row 0 reserved zero).
  Phase C: per window, one dma_gather per boundary side (a = last slot,
           b = before first slot) into double-buffered fp16 tiles, plus a
           self gather from h'; acc += A_w - B_w + S_w (fp16); then scale
           by dinv[dst], add bias, write out shard fp16 (host -> fp32).
"""

import numpy as np
from contextlib import ExitStack

import concourse.bacc as bacc
import concourse.bass as bass
import concourse.mybir as mybir
from concourse import ap_utils, bass_utils
from concourse.bass import MemorySpace
from concourse.library_config import mlp

D = 64


def configure(n=100000, ncores=8, wcap=25087, nw=4, slot=4, gch=32):
    # set problem geometry (module globals); defaults = real problem
    global N, NCORES, SHARD, WCAP, NW, WSTRIDE, TBL, ZROW
    global OUTROWS, NCOLS, AIT, AIT_REAL
    global L, SPC, GCH, NIDX, CLIM
    N = n
    NCORES = ncores
    SHARD = N // NCORES
    WCAP = wcap
    NW = nw
    WSTRIDE = WCAP + 1
    assert WSTRIDE % 128 == 0 and NW * WCAP >= N
    TBL = NW * WSTRIDE
    ZROW = WCAP
    OUTROWS = -(-SHARD // 128) * 128
    NCOLS = OUTROWS // 128
    assert TBL % 1024 == 0
    AIT = TBL // 1024
    max_row = (N - 1) + (N - 1) // WCAP
    AIT_REAL = -(-(max_row + 1) // 1024)
    L = slot
    SPC = 128 // L         # slot-prefix columns per chunk
    GCH = gch
    NIDX = GCH * 128
    # per-window staging table must stay int16-addressable
    CLIM = (32767 - 1) // SPC


configure()
LAST_NC = None


def dma_gather_raw(eng, out_ap, in_ap, idxs_ap, num_idxs, num_idxs_reg,
                   elem_size, elem_step, single_packet=False, queue_num=0):
    """bass.BassGpSimd.dma_gather minus the elem%256B assert (non-transpose
    HBM source; the ucode packetizes any elem size — only the table STRIDE
    must be a multiple of 256B). Verified on HW with 128B fp16 elems."""
    assert idxs_ap.dtype == mybir.dt.int16
    assert in_ap.dtype == out_ap.dtype
    assert in_ap.space == MemorySpace.DRAM
    assert idxs_ap.space == MemorySpace.SBUF
    assert out_ap.space == MemorySpace.SBUF
    assert ap_utils.ap_is_contiguous(in_ap.ap[1:])
    assert ap_utils.ap_is_contiguous(out_ap.ap[1:])
    assert ap_utils.ap_is_contiguous(idxs_ap.ap[1:])
    assert in_ap.ap[-1][1] == out_ap.ap[-1][1] == elem_size
    assert out_ap.ap[0][1] * out_ap.ap[1][1] == bass.round_up_to_multiple(
        num_idxs, 128)
    assert in_ap.ap[0][0] == elem_step
    stride_bytes = elem_step * mybir.dt.size(in_ap.dtype)
    stride_bytes_256 = bass.exact_div(stride_bytes, 256)
    assert stride_bytes_256 < 256
    return eng.add_instruction(
        mybir.InstDMAGatherAnt(
            name=eng.bass.get_next_instruction_name(),
            ins=[*eng.lower_ap_dma(in_ap, for_custom_bir_dma=True),
                 eng.lower_ap(idxs_ap),
                 eng.lower_val_access(eng.to_reg(num_idxs_reg))],
            outs=[eng.lower_ap(out_ap)],
            transpose=False,
            num_idxs=num_idxs,
            elem_size=elem_size,
            stride_bytes_256=stride_bytes_256,
            gen_mode=0,
            single_packet=single_packet,
            queue_num=queue_num,
            sbuf_tokens_per_rank=0,
            sbuf_free_dim_per_rank=0,
            sbuf_free_dim_pad_per_rank=0,
            sbuf_byte_offset=0,
        )
    )


def _wrap16(idx_flat):
    """Flat idx list -> dma_gather int16 wrap [16, n//16] tiled to 64."""
    n = idx_flat.shape[0]
    out = idx_flat.reshape(n // 16, 16).T.astype(np.int16)
    return np.tile(out, (4, 1))


def _pack_cell(counts, Lc):
    """Best-fit-ish decreasing pack of ceil(counts/Lc) slot runs into
    128-edge chunks (capacity 128//Lc slots)."""
    cap = 128 // Lc
    nz = np.nonzero(counts)[0]
    sizes = -(-counts[nz] // Lc)
    assert (sizes <= cap).all()
    smax = int(sizes.max()) if nz.size else 1
    by_size = [np.nonzero(sizes == s)[0] for s in range(smax + 1)]
    ptr = [0] * (smax + 1)
    remaining = int(nz.size)
    chunk_of = np.zeros(nz.size, np.int64)
    slot_start = np.zeros(nz.size, np.int64)
    cur = 0
    while remaining:
        space = cap
        while True:
            s = min(space, smax)
            while s > 0 and ptr[s] >= len(by_size[s]):
                s -= 1
            if s == 0:
                break
            k = by_size[s][ptr[s]]
            ptr[s] += 1
            chunk_of[k] = cur
            slot_start[k] = cap - space
            space -= s
            remaining -= 1
        cur += 1
    return nz, sizes, chunk_of, slot_start, cur


def _prep_core(src_g, dst_l):
    """Per-core, per-window gather/aggregation structures."""
    win = src_g // WCAP
    loc = src_g - win * WCAP  # 0..WCAP-1

    res = {"idx": [], "a": [], "b": [], "C": []}
    for w in range(NW):
        m = win == w
        dw = dst_l[m]
        lw = loc[m]
        order = np.argsort(dw, kind="stable")
        dw = dw[order]
        lw = lw[order]
        counts = np.bincount(dw, minlength=SHARD)
        nz, sizes, chunk_of, slot_start, C = _pack_cell(counts, L)
        assert C <= CLIM, f"cell needs {C} chunks > {CLIM}"
        starts = np.zeros(SHARD, np.int64)
        starts[1:] = np.cumsum(counts)[:-1]
        pos_base = chunk_of * 128 + slot_start * L
        idx = np.full(max(C, 1) * 128, ZROW, np.int64)
        within = np.arange(dw.shape[0]) - np.repeat(starts[nz], counts[nz])
        pos = np.repeat(pos_base, counts[nz]) + within
        idx[pos] = lw

        def sprow(ch, col):
            return (ch // GCH) * (GCH * SPC) + col * GCH + ch % GCH + 1

        a = np.zeros(SHARD, np.int64)
        b = np.zeros(SHARD, np.int64)
        a[nz] = sprow(chunk_of, slot_start + sizes - 1)
        sb0 = slot_start > 0
        bnz = np.zeros(nz.size, np.int64)
        bnz[sb0] = sprow(chunk_of[sb0], slot_start[sb0] - 1)
        b[nz] = bnz
        res["idx"].append(idx)
        res["a"].append(a)
        res["b"].append(b)
        res["C"].append(C)
    return res


def _build_program(C1, SPR, G1s, CTRUE):
    sched = [(w, gg) for w in range(NW) for gg in range(G1s[w])]

    def chg(w, gg):
        if gg == G1s[w] - 1:
            return CTRUE[w] - (G1s[w] - 1) * GCH
        return GCH
    NG = len(sched)             # total phase-B groups
    Gsum = [0]
    for w in range(NW):
        Gsum.append(Gsum[-1] + G1s[w])
    SPOFF = [0]                 # per-window staging table row offsets
    for w in range(NW):
        SPOFF.append(SPOFF[-1] + SPR[w])
    dt = mybir.dt
    TCOL = TBL // 128
    NAB = OUTROWS // 16         # idx cols per (window, a|b|s)
    IGC = NIDX // 16            # idx cols per phase-B group
    IWC = C1 * 8                # idx cols per window
    # h'-write iterations covering window w
    UPW = [min(-(-((w + 1) * WSTRIDE) // 1024), AIT_REAL) for w in range(NW)]
    # static interleave of phase-A iterations and phase-B groups on PE/DVE:
    # B-group g becomes eligible ~2 A-iters after its gather could start.
    ELIG = [min(UPW[gw] + 2 * (g - Gsum[gw]) + 2, AIT_REAL)
            for g, (gw, gg) in enumerate(sched)]
    MERGED = []
    _gq = 0
    for _it in range(AIT_REAL):
        MERGED.append(("A", _it))
        while _gq < len(sched) and ELIG[_gq] <= _it + 1:
            MERGED.append(("B", _gq))
            _gq += 1
    while _gq < len(sched):
        MERGED.append(("B", _gq))
        _gq += 1
    # consumed-op counters: per window ops (+A/-B or A-B), +S
    cnt_a = [1, 3, 6, 9][:NW]       # op index when A_w consumed
    cnt_b = [1, 4, 7, 10][:NW]      # op index when B_w consumed
    cnt_s = [2, 5, 8, 11][:NW]      # op index when S_w consumed

    nc = bacc.Bacc("TRN2", target_bir_lowering=False, debug=False,
                   num_devices=NCORES)
    t_xT = nc.dram_tensor("xT", [D, TBL], dt.float16, kind="ExternalInput")
    t_WT = nc.dram_tensor("WT", [D, D], dt.float16, kind="ExternalInput")
    t_SP = nc.dram_tensor("SP", [128, SPC], dt.float16,
                          kind="ExternalInput")
    t_degT = nc.dram_tensor("degT", [128, TCOL], dt.float32,
                            kind="ExternalInput")
    t_degD = nc.dram_tensor("degD", [128, NCOLS], dt.float32,
                            kind="ExternalInput")
    t_bBC = nc.dram_tensor("bBC", [128, D], dt.float32, kind="ExternalInput")
    t_idx1 = nc.dram_tensor("idx1", [NW, 64, IWC], dt.int16,
                            kind="ExternalInput")
    t_idxa = nc.dram_tensor("idxa", [64, NW * NAB], dt.int16,
                            kind="ExternalInput")
    t_idxb = nc.dram_tensor("idxb", [64, NW * NAB], dt.int16,
                            kind="ExternalInput")
    t_idxs = nc.dram_tensor("idxs", [64, NW * NAB], dt.int16,
                            kind="ExternalInput")
    t_hp = nc.dram_tensor("hp", [TBL, 128], dt.float16)
    t_sp = nc.dram_tensor("sp", [SPOFF[NW], 128], dt.float16)
    t_out = nc.dram_tensor("out_s", [OUTROWS, D], dt.float16,
                           kind="ExternalOutput")

    with ExitStack() as ctx:
        e = ctx.enter_context
        xb = [e(nc.sbuf_tensor(f"xb{i}", [D, 1024], dt.float16))
              for i in range(4)]
        hb = [e(nc.sbuf_tensor(f"hb{i}", [128, 512], dt.float16))
              for i in range(4)]
        WTs = e(nc.sbuf_tensor("WTs", [D, D], dt.float16))
        SPs = e(nc.sbuf_tensor("SPs", [128, SPC], dt.float16))
        bBCs = e(nc.sbuf_tensor("bBCs", [128, D], dt.float32))
        degTs = e(nc.sbuf_tensor("degTs", [128, TCOL], dt.float32))
        dinvTs = e(nc.sbuf_tensor("dinvTs", [128, TCOL], dt.float32))
        degDs = e(nc.sbuf_tensor("degDs", [128, NCOLS], dt.float32))
        dinvDs = e(nc.sbuf_tensor("dinvDs", [128, NCOLS], dt.float32))
        idx1s = e(nc.sbuf_tensor("idx1s", [64, 2 * IWC], dt.int16))
        idxas = e(nc.sbuf_tensor("idxas", [64, NW * NAB], dt.int16))
        idxbs = e(nc.sbuf_tensor("idxbs", [64, NW * NAB], dt.int16))
        idxss = e(nc.sbuf_tensor("idxss", [64, NW * NAB], dt.int16))
        msg = [e(nc.sbuf_tensor(f"msg{i}", [128, GCH, D], dt.float16))
               for i in range(4)]
        sps = [e(nc.sbuf_tensor(f"sps{i}", [SPC, GCH * D], dt.float16))
               for i in range(4)]
        zrow = e(nc.sbuf_tensor("zrow", [1, D], dt.float16))
        Ab = [e(nc.sbuf_tensor(f"Ab{i}", [128, NCOLS, D], dt.float16))
              for i in range(2)]
        Bb = [e(nc.sbuf_tensor(f"Bb{i}", [128, NCOLS, D], dt.float16))
              for i in range(2)]
        Sb = e(nc.sbuf_tensor("Sb", [128, NCOLS, D], dt.float16))
        accs = e(nc.sbuf_tensor("accs", [128, NCOLS, D], dt.float16))
        psum = [e(nc.psum_tensor(f"ps{i}", [128, 512], dt.float32))
                for i in range(8)]

        sW = e(nc.semaphore("sW"))
        sDeg = e(nc.semaphore("sDeg"))
        sLD3 = e(nc.semaphore("sLD3"))
        sAx = [e(nc.semaphore(f"sAx{i}")) for i in range(4)]
        sAmm = e(nc.semaphore("sAmm"))
        sAsc = e(nc.semaphore("sAsc"))
        sAout = [e(nc.semaphore(f"sAout{i}")) for i in range(4)]
        sDin = e(nc.semaphore("sDin"))
        sBidx = [e(nc.semaphore(f"sBidx{i}")) for i in range(2)]
        sBg = [e(nc.semaphore(f"sBg{i}")) for i in range(4)]
        sBmm = e(nc.semaphore("sBmm"))
        sBcpV = e(nc.semaphore("sBcpV"))
        sBcpS = e(nc.semaphore("sBcpS"))
        sBst = [e(nc.semaphore(f"sBst{i}")) for i in range(4)]
        sCz = e(nc.semaphore("sCz"))
        sCa = e(nc.semaphore("sCa"))
        sCb = e(nc.semaphore("sCb"))
        sCs = e(nc.semaphore("sCs"))
        sCacc = e(nc.semaphore("sCacc"))
        sFin = e(nc.semaphore("sFin"))

        def bcast(ap, reps):
            return bass.AP(ap.tensor, ap.offset, list(ap.ap) + [[0, reps]])

        with nc.Block() as block:

            @block.sync
            def _(sync: bass.BassEngine):
                # x tile 0 first so phase A starts immediately
                sync.dma_start(xb[0][:], t_xT[:, 0:1024]).then_inc(sAx[0], 16)
                sync.dma_start(WTs[:], t_WT[:]).then_inc(sW, 16)
                sync.dma_start(SPs[:], t_SP[:]).then_inc(sW, 16)
                sync.dma_start(degTs[:], t_degT[:]).then_inc(sDeg, 16)
                sync.dma_start(degDs[:], t_degD[:]).then_inc(sDeg, 16)
                sync.dma_start(idx1s[:, 0:G1s[0] * IGC],
                               t_idx1[0][:, :G1s[0] * IGC]
                               ).then_inc(sBidx[0], 32)
                sync.dma_start(idx1s[:, IWC:IWC + G1s[1] * IGC],
                               t_idx1[1][:, :G1s[1] * IGC]
                               ).then_inc(sBidx[1], 32)
                # phase A, interleaved x-in / h'-out
                for it in range(AIT_REAL + 2):
                    if 0 < it < AIT_REAL:
                        if it >= 4:
                            sync.wait_ge(sAmm, it - 3)
                        sync.dma_start(
                            xb[it % 4][:], t_xT[:, it * 1024:(it + 1) * 1024]
                        ).then_inc(sAx[it % 4], 16)
                    if it >= 2:
                        jo = it - 2
                        sync.wait_ge(sAsc, jo + 1)
                        src3 = hb[jo % 4][:].rearrange("p (c d) -> p c d",
                                                       d=D)
                        dst3 = bass.AP(t_hp, jo * 1024 * 128,
                                       [[128, 128], [128 * 128, 8], [1, D]])
                        sync.dma_start(dst3, src3).then_inc(sAout[jo % 4], 16)
                # final out (two column halves to shorten the tail)
                NH = NCOLS // 2
                sync.wait_ge(sCacc, 3 * NW - 4 + 5)
                o3a = bass.AP(t_out, 0, [[D, 128], [128 * D, NH], [1, D]])
                sync.dma_start(o3a, accs[:, 0:NH, :]).then_inc(sFin, 16)
                sync.wait_ge(sCacc, 3 * NW - 4 + 10)
                o3b = bass.AP(t_out, NH * 128 * D,
                              [[D, 128], [128 * D, NCOLS - NH], [1, D]])
                sync.dma_start(o3b, accs[:, NH:NCOLS, :]).then_inc(sFin, 16)
                sync.wait_ge(sFin, 33)

            @block.tensor
            def _(tensor):
                tensor.wait_ge(sW, 16)
                for kind, i in MERGED:
                    if kind == "A":
                        it = i
                        tensor.wait_ge(sAx[it % 4], (it // 4 + 1) * 16)
                        if it >= 4:
                            tensor.wait_ge(sAsc, it - 3)
                        for j in range(8):
                            ins = tensor.matmul(
                                psum[it % 4][:, j * D:(j + 1) * D],
                                xb[it % 4][:, j * 128:(j + 1) * 128],
                                WTs[:],
                                start=True, stop=True,
                            )
                        ins.then_inc(sAmm, 1)
                        continue
                    g = i
                    if g == 0:
                        tensor.wait_ge(sW, 32)
                    tensor.wait_ge(sBg[g % 4], (g // 4 + 1) * 16)
                    rhs = msg[g % 4][:].rearrange("p c d -> p (c d)")
                    for q in range(4):
                        if g >= 1:
                            tensor.wait_ge(sBcpS, 4 * (g - 1) + q + 1)
                        tensor.matmul(
                            psum[4 + q][:SPC, :],
                            SPs[:],
                            rhs[:, q * 512:(q + 1) * 512],
                            start=True, stop=True,
                        ).then_inc(sBmm, 1)

            @block.vector
            def _(vector):
                vector.wait_ge(sDeg, 32)
                vector.reciprocal(dinvTs[:], degTs[:]).then_inc(sDin, 1)
                vector.reciprocal(dinvDs[:], degDs[:]).then_inc(sDin, 1)
                vector.wait_ge(sDin, 3)  # sqrt done on scalar
                cnt = 0

                def accum(w):
                    nonlocal cnt
                    if w < NW - 1:
                        vector.wait_ge(sCa, (w + 1) * 32)
                        if w == 0:
                            vector.wait_ge(sCb, 32)
                            vector.tensor_tensor(
                                accs[:], Ab[0][:], Bb[0][:],
                                op=mybir.AluOpType.subtract,
                            ).then_inc(sCacc, 1)
                            cnt += 1
                        else:
                            vector.wait_ge(sCacc, cnt)
                            vector.tensor_tensor(
                                accs[:], accs[:], Ab[w % 2][:],
                                op=mybir.AluOpType.add,
                            ).then_inc(sCacc, 1)
                            cnt += 1
                            vector.wait_ge(sCb, (w + 1) * 32)
                            vector.wait_ge(sCacc, cnt)
                            vector.tensor_tensor(
                                accs[:], accs[:], Bb[w % 2][:],
                                op=mybir.AluOpType.subtract,
                            ).then_inc(sCacc, 1)
                            cnt += 1
                        vector.wait_ge(sCs, (w + 1) * 32)
                        vector.wait_ge(sCacc, cnt)
                        vector.tensor_tensor(
                            accs[:], accs[:], Sb[:], op=mybir.AluOpType.add
                        ).then_inc(sCacc, 1)
                        cnt += 1
                        return
                    # last window: half-split accumulate + finale
                    NH = NCOLS // 2
                    for h in range(2):
                        c0 = h * NH
                        c1 = NCOLS if h else NH
                        base = w * 32 + (h + 1) * 16
                        ah = accs[:, c0:c1, :]
                        vector.wait_ge(sCa, base)
                        vector.wait_ge(sCacc, cnt)
                        vector.tensor_tensor(
                            ah, ah, Ab[w % 2][:, c0:c1, :],
                            op=mybir.AluOpType.add,
                        ).then_inc(sCacc, 1)
                        cnt += 1
                        vector.wait_ge(sCb, base)
                        vector.wait_ge(sCacc, cnt)
                        vector.tensor_tensor(
                            ah, ah, Bb[w % 2][:, c0:c1, :],
                            op=mybir.AluOpType.subtract,
                        ).then_inc(sCacc, 1)
                        cnt += 1
                        vector.wait_ge(sCs, base)
                        vector.wait_ge(sCacc, cnt)
                        vector.tensor_tensor(
                            ah, ah, Sb[:, c0:c1, :],
                            op=mybir.AluOpType.add,
                        ).then_inc(sCacc, 1)
                        cnt += 1
                        dvDh = bcast(dinvDs[:, c0:c1], D)
                        vector.wait_ge(sCacc, cnt)
                        vector.tensor_tensor(
                            ah, ah, dvDh, op=mybir.AluOpType.mult
                        ).then_inc(sCacc, 1)
                        cnt += 1
                        apb = bBCs[:]
                        bbh = bass.AP(apb.tensor, apb.offset,
                                      [list(apb.ap[0]), [0, c1 - c0],
                                       list(apb.ap[1])])
                        vector.wait_ge(sLD3, 64)
                        vector.wait_ge(sCacc, cnt)
                        vector.tensor_tensor(
                            ah, ah, bbh, op=mybir.AluOpType.add
                        ).then_inc(sCacc, 1)
                        cnt += 1

                for it in range(AIT_REAL):
                    vector.wait_ge(sAmm, it + 1)
                    if it >= 4:
                        vector.wait_ge(sAout[it % 4],
                                       ((it - 4) // 4 + 1) * 16)
                    ps3 = psum[it % 4][:].rearrange("p (c d) -> p c d", d=D)
                    hb3 = hb[it % 4][:].rearrange("p (c d) -> p c d", d=D)
                    dv = bcast(dinvTs[:, it * 8:(it + 1) * 8], D)
                    vector.tensor_tensor(
                        hb3, ps3, dv, op=mybir.AluOpType.mult
                    ).then_inc(sAsc, 1)
                for w in range(NW):
                    accum(w)

            @block.scalar
            def _(scalar):
                scalar.wait_ge(sDin, 2)
                scalar.activation(dinvTs[:], dinvTs[:],
                                  mybir.ActivationFunctionType.Sqrt)
                scalar.activation(dinvDs[:], dinvDs[:],
                                  mybir.ActivationFunctionType.Sqrt
                                  ).then_inc(sDin, 1)
                # phase-C index tables + bias (needed from phase_c(0) on)
                scalar.dma_start(idxas[:], t_idxa[:]).then_inc(sLD3, 16)
                scalar.dma_start(idxbs[:], t_idxb[:]).then_inc(sLD3, 16)
                scalar.dma_start(idxss[:], t_idxs[:]).then_inc(sLD3, 16)
                scalar.dma_start(bBCs[:], t_bBC[:]).then_inc(sLD3, 16)
                # zero rows of the slot-prefix tables
                scalar.wait_ge(sFin, 1)
                for w in range(NW):
                    scalar.dma_start(
                        bass.AP(t_sp, SPOFF[w] * 128, [[128, 1], [1, D]]),
                        zrow[:]).then_inc(sCz, 16)
                gload_at = {Gsum[w]: w + 1 for w in range(1, NW - 1)}
                for g, (gw, gg) in enumerate(sched):
                    if g in gload_at:
                        # prefetch idx for window g+1 one window ahead;
                        # scalar position implies the target buffer's old
                        # readers (window wl-2 gathers) are long done.
                        wl = gload_at[g]
                        scalar.dma_start(
                            idx1s[:, (wl % 2) * IWC:
                                  (wl % 2) * IWC + G1s[wl] * IGC],
                            t_idx1[wl][:, :G1s[wl] * IGC],
                        ).then_inc(sBidx[wl % 2], 32)
                    if g >= 4:
                        scalar.wait_ge(sBst[g % 4], ((g - 4) // 4 + 1) * 16)
                    for q in range(4):
                        scalar.wait_ge(sBmm, 4 * g + q + 1)
                        scalar.activation(
                            sps[g % 4][:, q * 512:(q + 1) * 512],
                            psum[4 + q][:SPC, :],
                            mybir.ActivationFunctionType.Copy,
                        ).then_inc(sBcpS, 1)

            @block.gpsimd
            def _(gpsimd: bass.BassGpSimd):
                gpsimd.load_library(mlp)
                gpsimd.memset(zrow[:], 0.0).then_inc(sFin, 1)

                def staging(gs):
                    if gs < 0 or gs >= NG:
                        return
                    sgw, sgg = sched[gs]
                    nch = chg(sgw, sgg)
                    gpsimd.wait_ge(sBcpS, 4 * gs + 4)
                    src3 = sps[gs % 4][:].rearrange("s (j d) -> s j d",
                                                    d=D)[:, 0:nch, :]
                    dst3 = bass.AP(
                        t_sp,
                        (SPOFF[sgw] + 1 + sgg * GCH * SPC) * 128,
                        [[GCH * 128, SPC], [128, nch], [1, D]],
                    )
                    gpsimd.dma_start(dst3, src3).then_inc(sBst[gs % 4], 16)

                def phase_c(w):
                    # phase C for window w (after its staging lands)
                    for i in range(4):
                        gpsimd.wait_ge(sBst[i],
                                       ((Gsum[w + 1] + 3 - i) // 4) * 16)
                    if w == 0:
                        gpsimd.wait_ge(sLD3, 48)
                        gpsimd.wait_ge(sCz, NW * 16)
                    spw = bass.AP(t_sp, SPOFF[w] * 128,
                                  [[128, SPR[w]], [1, D]])
                    hp_w = bass.AP(t_hp, w * WSTRIDE * 128,
                                   [[128, WSTRIDE], [1, D]])
                    NABH = NAB // 2
                    NOH = OUTROWS // 2
                    for h in range(2):
                        c0 = h * (NCOLS // 2)
                        c1 = NCOLS if h else NCOLS // 2
                        if w >= 2 and h == 0:
                            gpsimd.wait_ge(sCacc, cnt_a[w - 2])
                        dma_gather_raw(
                            gpsimd, Ab[w % 2][:, c0:c1, :], spw,
                            idxas[:, w * NAB + h * NABH:
                                  w * NAB + (h + 1) * NABH],
                            NOH, NOH, D, 128, single_packet=False,
                        ).then_inc(sCa, 16)
                        if w >= 2 and h == 0:
                            gpsimd.wait_ge(sCacc, cnt_s[w - 2])
                        dma_gather_raw(
                            gpsimd, Bb[w % 2][:, c0:c1, :], spw,
                            idxbs[:, w * NAB + h * NABH:
                                  w * NAB + (h + 1) * NABH],
                            NOH, NOH, D, 128, single_packet=False,
                        ).then_inc(sCb, 16)
                        # self gather from h' window w
                        if w >= 1 and h == 0:
                            gpsimd.wait_ge(sCacc, cnt_s[w - 1])
                        dma_gather_raw(
                            gpsimd, Sb[:, c0:c1, :], hp_w,
                            idxss[:, w * NAB + h * NABH:
                                  w * NAB + (h + 1) * NABH],
                            NOH, NOH, D, 128, single_packet=False,
                        ).then_inc(sCs, 16)

                g = 0
                for w in range(NW):
                    # h' rows of window w ready
                    up = UPW[w]
                    for i in range(4):
                        gpsimd.wait_ge(sAout[i], ((up + 3 - i) // 4) * 16)
                    gpsimd.wait_ge(sBidx[w % 2], (w // 2 + 1) * 32)
                    ib = (w % 2) * IWC
                    hp_w = bass.AP(t_hp, w * WSTRIDE * 128,
                                   [[128, WSTRIDE], [1, D]])
                    for gg in range(G1s[w]):
                        if g >= 4:
                            gpsimd.wait_ge(sBmm, 4 * (g - 4) + 4)
                        nch = chg(w, gg)
                        dma_gather_raw(
                            gpsimd,
                            msg[g % 4][:, 0:nch, :], hp_w,
                            idx1s[:, ib + gg * IGC:
                                  ib + gg * IGC + nch * 8],
                            nch * 128, nch * 128, D, 128,
                            single_packet=False,
                        ).then_inc(sBg[g % 4], 16)
                        g += 1
                        staging(g - 3)
                        # phase C for the PREVIOUS window, well into this
                        # window: its staging has landed and the gather
                        # backlog covers phase C's SWDGE generation latency.
                        if w >= 1 and gg == min(4, G1s[w] - 1):
                            phase_c(w - 1)
                for gs in range(max(NG - 3, 0), NG):
                    staging(gs)
                phase_c(NW - 1)

        nc.compile()
    return nc


def prepare(x, edge_index, W, b):
    x = np.asarray(x, dtype=np.float32)
    edge_index = np.asarray(edge_index)
    W = np.asarray(W, dtype=np.float32)
    b = np.asarray(b, dtype=np.float32)

    src = np.asarray(edge_index[0], dtype=np.int64)
    dst = np.asarray(edge_index[1], dtype=np.int64)

    deg = (np.bincount(dst, minlength=N) + 1.0).astype(np.float32)

    tbl_row = np.arange(N) + np.arange(N) // WCAP
    x_table = np.zeros((TBL, D), np.float16)
    x_table[tbl_row] = x.astype(np.float16)
    xT = np.ascontiguousarray(x_table.T)
    deg_table = np.ones(TBL, np.float32)
    deg_table[tbl_row] = deg
    degT = np.ascontiguousarray(deg_table.reshape(TBL // 128, 128).T)

    WT = np.ascontiguousarray(W.T.astype(np.float16))
    SP = (np.arange(128)[:, None] <= (L * np.arange(SPC) + L - 1)[None, :]
          ).astype(np.float16)
    bBC = np.ascontiguousarray(np.broadcast_to(b, (128, D))).astype(
        np.float32)

    shard_of = dst // SHARD
    order_c = np.argsort(shard_of, kind="stable")
    bounds = np.searchsorted(shard_of[order_c], np.arange(NCORES + 1))
    cores = []
    for c in range(NCORES):
        sel = order_c[bounds[c]:bounds[c + 1]]
        cores.append(_prep_core(src[sel], dst[sel] - c * SHARD))

    C1s = []
    CTRUE = []
    for w in range(NW):
        cw = max(cr["C"][w] for cr in cores)
        CTRUE.append(max(cw, 1))
        C1s.append(max(GCH, -(-cw // GCH) * GCH))
    C1 = max(C1s)
    G1s = [c // GCH for c in C1s]
    SPR = [1 + c * SPC for c in C1s]
    assert max(SPR) <= 32767, f"slot-prefix table too large: {max(SPR)}"

    NAB = OUTROWS // 16
    in_maps = []
    for c in range(NCORES):
        cr = cores[c]
        idx1 = np.empty((NW, 64, C1 * 8), np.int16)
        idxa = np.empty((64, NW * NAB), np.int16)
        idxb = np.empty((64, NW * NAB), np.int16)
        idxs = np.empty((64, NW * NAB), np.int16)
        vown = np.arange(c * SHARD, (c + 1) * SHARD)
        rown = vown + vown // WCAP
        wown = rown // WSTRIDE
        for w in range(NW):
            fl = np.full(C1 * 128, ZROW, np.int64)
            fl[:cr["idx"][w].shape[0]] = cr["idx"][w]
            parts = [_wrap16(fl[g * NIDX:(g + 1) * NIDX])
                     for g in range(C1 // GCH)]
            idx1[w] = np.concatenate(parts, axis=1)
            for arr, dest in ((cr["a"][w], idxa), (cr["b"][w], idxb)):
                af = np.zeros(OUTROWS, np.int64)
                af[:SHARD] = arr
                dest[:, w * NAB:(w + 1) * NAB] = _wrap16(af)
            sf = np.full(OUTROWS, ZROW, np.int64)
            inw = wown == w
            sf[:SHARD][inw] = rown[inw] - w * WSTRIDE
            idxs[:, w * NAB:(w + 1) * NAB] = _wrap16(sf)
        degD_flat = np.ones(OUTROWS, np.float32)
        degD_flat[:SHARD] = deg[c * SHARD:(c + 1) * SHARD]
        degD = np.ascontiguousarray(degD_flat.reshape(NCOLS, 128).T)
        in_maps.append({
            "xT": xT, "WT": WT, "SP": SP, "degT": degT, "degD": degD,
            "bBC": bBC, "idx1": idx1, "idxa": idxa, "idxb": idxb,
            "idxs": idxs,
        })

    global SPR_G
    SPR_G = SPR
    nc = _build_program(C1, SPR, G1s, CTRUE)
    global LAST_NC
    LAST_NC = nc
    return nc, in_maps


def kernel(x, edge_index, W, b):
    nc, in_maps = prepare(x, edge_index, W, b)
    res = bass_utils.run_bass_kernel_spmd(nc, in_maps,
                                          core_ids=list(range(NCORES)))
    out = np.empty((N, D), np.float32)
    for c in range(NCORES):
        out[c * SHARD:(c + 1) * SHARD] = \
            res.results[c]["out_s"][:SHARD].astype(np.float32)
    return out
